# revision 14
# baseline (speedup 1.0000x reference)
"""Self-contained GCN edge-dot kernel for 8 TRN2 NeuronCores (v2).

kernel(**inputs) takes the FULL problem inputs and returns sigmoid edge
scores for every edge, computed SPMD across 8 cores with bass/bacc.

Strategy vs v1: nodes assigned degree-balanced to (core, block) with a
piece-major gather-table numbering so the two inter-layer exchanges
(P2 = H1 @ W_pass2 and H2, both 64-wide bf16 packed two-nodes-per-256B
row) stream as per-piece AllGather collectives overlapped with compute;
one-hot selection matrices built per group on DVE/Pool via
tensor_scalar (is_equal x val, 4x DVE mode); phase-3 edge dot uses
PE transpose + batched ACT psum copy + batched DVE mult/reduce; edge
val folded into the one-hot so gathered rows are used unscaled.
"""
import sys
sys.path.insert(0, "/opt/trn_rl_repo")
import numpy as np
import ml_dtypes
import concourse.bass as bass
import concourse.bacc as bacc
import concourse.mybir as mybir
from concourse import masks
from concourse.bass_utils import run_bass_kernel_spmd

F32 = mybir.dt.float32
BF16 = mybir.dt.bfloat16
I16 = mybir.dt.int16
AF = mybir.ActivationFunctionType
ALU = mybir.AluOpType
NCORES = 8


# ---------------------------------------------------------------- host planning
class Plan:
    pass


def plan_graph(edge_row, edge_col, edge_vals, n_nodes, NB=49, CB=4,
               pstart=(0, 25)):
    p = Plan()
    NPc = NB * 128
    NP = NPc * NCORES
    assert n_nodes <= NP
    NPIECES = len(pstart)
    pstart = list(pstart) + [NB]
    pn = [pstart[k + 1] - pstart[k] for k in range(NPIECES)]
    p.NB, p.NPc, p.NP, p.CB, p.NPIECES = NB, NPc, NP, CB, NPIECES
    p.pn = pn

    E = len(edge_row)
    deg = np.bincount(edge_row, minlength=NP)
    order = np.argsort(-deg, kind="stable")
    nblocks = NCORES * NB
    newpos = np.empty(NP, np.int64)
    for g in range(nblocks):
        members = order[g::nblocks]
        c, b = g // NB, g % NB
        newpos[members] = c * NPc + b * 128 + np.arange(len(members))
    p.newpos = newpos
    perm = np.empty(NP, np.int64)
    perm[newpos] = np.arange(NP)
    p.perm = perm

    # piece-major gather-table row numbering: trow(c,b,l) for piece k
    #   = trowbase[k] + c*pn[k]*128 + (b - pstart[k])*128 + l
    trowbase = np.zeros(NPIECES + 1, np.int64)
    for k in range(NPIECES):
        trowbase[k + 1] = trowbase[k] + NCORES * pn[k] * 128
    p.trowbase = trowbase
    pos_c = newpos // NPc
    pos_b = (newpos % NPc) // 128
    pos_l = newpos % 128
    pn_arr = np.array(pn)
    pstart_arr = np.array(pstart[:NPIECES])
    pk = np.searchsorted(pstart_arr[1:] if NPIECES > 1 else np.array([NB]),
                         pos_b, side="right")
    pk = np.minimum(pk, NPIECES - 1)
    trow_of_node = (trowbase[pk] + pos_c * pn_arr[pk] * 128 +
                    (pos_b - pstart_arr[pk]) * 128 + pos_l)
    p.trow_of_node = trow_of_node  # original node id -> table row
    # inverse: table row -> original node id
    node_of_trow = np.empty(NP, np.int64)
    node_of_trow[trow_of_node] = np.arange(NP)
    p.node_of_trow = node_of_trow

    nr = newpos[edge_row]          # dest in (c,b,l) space
    core = nr // NPc
    blk = (nr % NPc) // 128
    dloc = nr % 128
    strow = trow_of_node[edge_col]  # source table row
    spiece = np.searchsorted(trowbase[1:NPIECES + 1], strow, side="right")
    sidx1 = strow - trowbase[spiece]            # idx within piece (p1, rows)
    sidx2 = sidx1 // 2                          # idx within piece (p2/p3 pairs)
    par = (strow % 2).astype(np.int64)          # parity within pair row

    # bucket (core, block, piece) -> even/odd edge lists
    G = np.zeros((NB, NPIECES), np.int64)
    buckets = {}
    for c in range(NCORES):
        m_c = core == c
        for b in range(NB):
            m_b = m_c & (blk == b)
            for k in range(NPIECES):
                m = m_b & (spiece == k)
                ev_ = np.nonzero(m & (par == 0))[0]
                od_ = np.nonzero(m & (par == 1))[0]
                buckets[(c, b, k)] = (ev_, od_)
                ng = max(-(-len(ev_) // 64), -(-len(od_) // 64))
                G[b, k] = max(G[b, k], ng)
    for b in range(NB):
        if G[b].sum() == 0:
            G[b, 0] = 1
    p.G = G
    p.Gtot = int(G.sum())
    S = p.Gtot * 128

    chunks = [list(range(i, min(i + CB, NB))) for i in range(0, NB, CB)]
    p.chunks = chunks
    segs = []
    gidx = 0
    firstg = {}
    lastg = {}
    for ci, cblocks in enumerate(chunks):
        for k in range(NPIECES):
            for b in cblocks:
                ng = int(G[b, k])
                if ng == 0:
                    continue
                segs.append((ci, k, b, gidx, ng))
                if b not in firstg:
                    firstg[b] = gidx
                lastg[b] = gidx + ng - 1
                gidx += ng
    assert gidx == p.Gtot
    p.segs = segs
    p.firstg, p.lastg = firstg, lastg
    # gather segments: one dma_gather per (ci, k) covering its blocks' groups
    gsegs = []
    i = 0
    while i < len(segs):
        ci, k, b, g0, ng = segs[i]
        j = i
        tot = 0
        while j < len(segs) and segs[j][0] == ci and segs[j][1] == k:
            tot += segs[j][4]
            j += 1
        gsegs.append((ci, k, g0, tot))
        i = j
    p.gsegs = gsegs
    p.GH = max(t for (_, _, _, t) in gsegs)
    p.block_of_g = {}
    for (ci, k, b, g0, ng) in segs:
        for g in range(g0, g0 + ng):
            p.block_of_g[g] = b

    p.idx1 = np.zeros((NCORES, S), np.int16)
    p.idx2 = np.zeros((NCORES, S), np.int16)
    p.dloc = np.zeros((NCORES, S), np.float32)
    p.val = np.zeros((NCORES, S), np.float32)
    p.parmask = (np.arange(128) < 64).astype(np.float32)
    p.slot_of_edge = np.full(E, -1, np.int64)
    p.core_of_edge = core
    for c in range(NCORES):
        for (ci, k, b, g0, ng) in segs:
            ev_, od_ = buckets[(c, b, k)]
            for half, eids in ((0, ev_), (1, od_)):
                n = len(eids)
                assert n <= ng * 64
                gi = np.arange(n) // 64
                sl = (g0 + gi) * 128 + half * 64 + np.arange(n) % 64
                p.idx1[c, sl] = sidx1[eids]
                p.idx2[c, sl] = sidx2[eids]
                p.dloc[c, sl] = dloc[eids]
                p.val[c, sl] = edge_vals[eids]
                p.slot_of_edge[eids] = sl
    return p


def wrap_idx(idx_flat):
    S = len(idx_flat)
    w = idx_flat.reshape(S // 16, 16).T
    return np.tile(w, (8, 1)).copy()


def colmajor(a):
    S = len(a)
    return a.reshape(S // 128, 128).T.copy()


# ---------------------------------------------------------------- emission fw
class Counters:
    def __init__(self):
        self.val = {}
        self.last = {}

    def inc(self, sem, by):
        self.val[sem] = self.val.get(sem, 0) + by
        return self.val[sem]

    def cur(self, sem):
        return self.val.get(sem, 0)

    def wait(self, eng_ops, eng_name, sem, v):
        if v <= 0:
            return
        key = (eng_name, sem)
        if self.last.get(key, -1) >= v:
            return
        self.last[key] = v
        eng_ops.append(("wait", sem, v))


DMA, V, A, P, PL, CC = "dma", "v", "a", "p", "pl", "cc"
GTS = ("gt0", "gt1", "gt2", "gt3")
WRS = ("wr0", "wr1")


def build(plan):
    p = plan
    NB, NPc, NP, CB = p.NB, p.NPc, p.NP, p.CB
    NPIECES, pn, G = p.NPIECES, p.pn, p.G
    Gtot, segs, gsegs, chunks = p.Gtot, p.segs, p.gsegs, p.chunks
    firstg, lastg, block_of_g = p.firstg, p.lastg, p.block_of_g
    S = Gtot * 128
    GH = p.GH
    NSLOT = 4      # gather buffer slots
    OHS = 16       # one-hot slots

    nc = bacc.Bacc()
    dp = nc.declare_dram_parameter
    xg = dp("xg", [NP, 128], BF16, isOutput=False)
    xlT_in = dp("xlT", [128, NPc], BF16, isOutput=False)
    idx1_in = dp("idx1", [128, S // 16], I16, isOutput=False)
    idx2_in = dp("idx2", [128, S // 16], I16, isOutput=False)
    dloc_in = dp("dloc", [128, Gtot], F32, isOutput=False)
    val_in = dp("val", [128, Gtot], F32, isOutput=False)
    vale_in = dp("vale", [128, Gtot], F32, isOutput=False)
    valo_in = dp("valo", [128, Gtot], F32, isOutput=False)
    w1p_in = dp("w1p", [128, 128], BF16, isOutput=False)
    w1s_in = dp("w1s", [128, 128], BF16, isOutput=False)
    w2p_in = dp("w2p", [128, 64], BF16, isOutput=False)
    w2s_in = dp("w2s", [128, 64], BF16, isOutput=False)
    b1_in = dp("b1", [128, 1], F32, isOutput=False)
    b2_in = dp("b2rep", [128, 64], F32, isOutput=False)
    sx_out = dp("sx", [128, Gtot], F32, isOutput=True)

    p2_loc = nc.dram_tensor("p2_loc", [NPc, 64], BF16)
    h2_loc = nc.dram_tensor("h2_loc", [NPc, 64], BF16)
    p2t = nc.dram_tensor("p2t", [NP // 2, 128], BF16, addr_space="Shared")
    h2t = nc.dram_tensor("h2t", [NP // 2, 128], BF16, addr_space="Shared")

    # piece boundaries in table spaces
    trowbase = p.trowbase
    pairbase = [int(trowbase[k] // 2) for k in range(NPIECES + 1)]
    locbase = []   # local p2_loc/h2_loc row range per piece
    acc = 0
    for k in range(NPIECES):
        locbase.append(acc)
        acc += pn[k] * 128

    ops = {e: [] for e in ("sp", "pool", "dve", "act", "pe")}
    C = Counters()
    ev = {}
    sp, pool, dve, act, pe = (ops[k] for k in ("sp", "pool", "dve", "act", "pe"))

    # ---- loads
    _ld_names = ("idx1", "idx2", "dloc", "val", "vale", "valo", "xlT",
                 "w1p", "w1s", "w2p", "w2s", "b1", "b2")
    for name in _ld_names:
        sp.append(("dma_sb", name))
        C.inc(DMA, 16)
    for name in _ld_names:
        ev["ld_" + name] = (DMA, C.cur(DMA))
    pool.append(("iota",))
    pool.append(("ident",))
    ev["p0_pool"] = (PL, C.inc(PL, 1))

    # last p1/p2 block using psum bank j (for bank-reuse waits)
    last_user = {}
    for b in range(NB):
        last_user[b % CB] = b
    pstart_list = [0]
    for k in range(NPIECES - 1):
        pstart_list.append(pstart_list[-1] + pn[k])
    piece_of_block = {}
    for b in range(NB):
        piece_of_block[b] = max(k for k in range(NPIECES)
                                if pstart_list[k] <= b)

    # ---------------- gather emission helper
    gseq = [0]

    def emit_gather(ph, ci, k, g0, gsum):
        s = gseq[0]
        slot = s % NSLOT
        # slot reuse: wait consumers of gather s-NSLOT
        prev = s - NSLOT
        if prev >= 0:
            pph, pg0, pgsum = gmeta[prev]
            if pph == "p3":
                C.wait(pool, "pool", V, ev[f"p3_multdone_g{pg0 + pgsum - 1}"][1])
            else:
                C.wait(pool, "pool", P, ev[f"{pph}_agg_g{pg0 + pgsum - 1}"][1])
        C.wait(pool, "pool", DMA, ev["ld_idx1" if ph == "p1" else "ld_idx2"][1])
        if ph == "p2":
            C.wait(pool, "pool", CC, k + 1)
        elif ph == "p3":
            C.wait(pool, "pool", CC, NPIECES + k + 1)
        pool.append(("gather", ph, k, g0, gsum, slot))
        gs = GTS[slot]
        ev[f"{ph}_gather_g{g0}"] = (gs, C.inc(gs, 16))
        gmeta.append((ph, g0, gsum))
        gseq[0] += 1
        return slot

    gmeta = []

    # ---------------- p1 / p2 aggregation phases
    def emit_agg_phase(ph):
        for (ci, k, g0s, gsum) in gsegs:
            slot = emit_gather(ph, ci, k, g0s, gsum)
            segs_here = [(b, g0, ng) for (ci2, k2, b, g0, ng) in segs
                         if ci2 == ci and k2 == k]
            for (b, g0, ng) in segs_here:
                for g in range(g0, g0 + ng):
                    # weighted one-hot build(s)
                    if ph == "p1":
                        C.wait(dve, "dve", DMA, ev["ld_val"][1])
                        C.wait(dve, "dve", PL, ev["p0_pool"][1])
                        if g - OHS >= 0:
                            C.wait(dve, "dve", P, ev[f"p1_agg_g{g - OHS}"][1])
                        dve.append(("ohw", g, "val", 0))
                        ev[f"p1_oh_g{g}"] = (V, C.inc(V, 1))
                    else:
                        # two parity-masked builds: even on DVE, odd on POOL
                        # p2 uses oh slots (2g)%OHS and (2g+1)%OHS
                        lue = (2 * g) % OHS + OHS * ((Gtot - 1 - (2 * g) % OHS) // OHS)
                        luo = (2 * g + 1) % OHS + OHS * ((Gtot - 1 - (2 * g + 1) % OHS) // OHS)
                        C.wait(dve, "dve", DMA, ev["ld_vale"][1])
                        C.wait(dve, "dve", P, ev[f"p1_agg_g{lue}"][1])
                        if g - OHS // 2 >= 0:
                            C.wait(dve, "dve", P, ev[f"p2_agg_g{g - OHS // 2}"][1])
                        dve.append(("ohw", g, "vale", 0))
                        ev[f"p2_ohe_g{g}"] = (V, C.inc(V, 1))
                        C.wait(pool, "pool", DMA, ev["ld_valo"][1])
                        C.wait(pool, "pool", P, ev[f"p1_agg_g{luo}"][1])
                        if g - OHS // 2 >= 0:
                            C.wait(pool, "pool", P, ev[f"p2_agg_g{g - OHS // 2}"][1])
                        pool.append(("ohw", g, "valo", 1))
                        ev[f"p2_oho_g{g}"] = (PL, C.inc(PL, 1))

                    # PE: aggregation matmul(s)
                    _gs, _gv = ev[f"{ph}_gather_g{g0s}"]
                    C.wait(pe, "pe", _gs, _gv)
                    if ph == "p1":
                        C.wait(pe, "pe", V, ev[f"p1_oh_g{g}"][1])
                    else:
                        C.wait(pe, "pe", V, ev[f"p2_ohe_g{g}"][1])
                        C.wait(pe, "pe", PL, ev[f"p2_oho_g{g}"][1])
                    first = g == firstg[b]
                    last = g == lastg[b]
                    if first:
                        # psum bank reuse
                        if ph == "p1":
                            if b - CB >= 0:
                                C.wait(pe, "pe", A, ev[f"p1_aggcopy_b{b - CB}"][1])
                        else:
                            lb = last_user[b % CB]
                            C.wait(pe, "pe", A, ev[f"p1_aggcopy_b{lb}"][1])
                            if b - CB >= 0:
                                C.wait(pe, "pe", V, ev[f"p2_h2add_b{b - CB}"][1])
                    j = g - g0s
                    if ph == "p1":
                        pe.append(("agg1", b, g, j, slot, first, last))
                        ev[f"p1_agg_g{g}"] = (P, C.inc(P, 1))
                    else:
                        pe.append(("agg2", b, g, j, slot, first, last))
                        ev[f"p2_agg_g{g}"] = (P, C.inc(P, 2))
                    if last:
                        ev[f"{ph}_aggstop_b{b}"] = (P, C.cur(P))
                        if ph == "p1":
                            p1_block_tail(b)
                        else:
                            p2_block_tail(b)

    def p1_block_tail(b):
        # ACT: aggcopy psum->sbuf bf16
        C.wait(act, "act", P, ev[f"p1_aggstop_b{b}"][1])
        if b - 2 >= 0:
            C.wait(act, "act", P, ev[f"p1_h1mm_b{b - 2}"][1])
        act.append(("aggcopy", b))
        ev[f"p1_aggcopy_b{b}"] = (A, C.inc(A, 1))
        # PE: h1 = relu(W1p^T aggT + W1s^T xlT + b1)
        C.wait(pe, "pe", DMA, ev["ld_xlT"][1])
        C.wait(pe, "pe", A, ev[f"p1_aggcopy_b{b}"][1])
        if b - 1 >= 0:
            C.wait(pe, "pe", A, ev[f"p1_h1relu_b{b - 1}"][1])
        pe.append(("h1mm", b))
        ev[f"p1_h1mm_b{b}"] = (P, C.inc(P, 2))
        C.wait(act, "act", P, ev[f"p1_h1mm_b{b}"][1])
        C.wait(act, "act", DMA, ev["ld_b1"][1])
        act.append(("h1relu", b))
        ev[f"p1_h1relu_b{b}"] = (A, C.inc(A, 1))
        # PE: P2 block + S2 block
        C.wait(pe, "pe", DMA, ev["ld_w2s"][1])
        C.wait(pe, "pe", A, ev[f"p1_h1relu_b{b}"][1])
        if b - 1 >= 0:
            C.wait(pe, "pe", A, ev[f"p1_s2cp_b{b - 1}"][1])
        pe.append(("p2mm", b))
        ev[f"p1_p2mm_b{b}"] = (P, C.inc(P, 2))
        C.wait(act, "act", P, ev[f"p1_p2mm_b{b}"][1])
        act.append(("p2cp", b))
        ev[f"p1_p2cp_b{b}"] = (A, C.inc(A, 1))
        act.append(("s2cp", b))
        ev[f"p1_s2cp_b{b}"] = (A, C.inc(A, 1))
        C.wait(sp, "sp", A, ev[f"p1_p2cp_b{b}"][1])
        wk = WRS[piece_of_block[b]]
        C.wait(sp, "sp", wk, C.cur(wk))
        sp.append(("p2wr", b))
        ev[f"p1_p2wr_b{b}"] = (wk, C.inc(wk, 16))

    def p2_block_tail(b):
        C.wait(dve, "dve", P, ev[f"p2_aggstop_b{b}"][1])
        C.wait(dve, "dve", V, ev["s2bias"][1])
        if b - 2 >= 0:
            C.wait(dve, "dve", A, ev[f"p2_h2relu_b{b - 2}"][1])
        dve.append(("h2add", b))
        ev[f"p2_h2add_b{b}"] = (V, C.inc(V, 1))
        C.wait(act, "act", V, ev[f"p2_h2add_b{b}"][1])
        act.append(("h2relu", b))
        ev[f"p2_h2relu_b{b}"] = (A, C.inc(A, 1))
        C.wait(sp, "sp", A, ev[f"p2_h2relu_b{b}"][1])
        wk = WRS[piece_of_block[b]]
        C.wait(sp, "sp", wk, C.cur(wk))
        sp.append(("h2wr", b))
        ev[f"p2_h2wr_b{b}"] = (wk, C.inc(wk, 16))

    # ================= PHASE 1 =================
    emit_agg_phase("p1")

    # s2bias after all s2cp
    C.wait(dve, "dve", A, ev[f"p1_s2cp_b{NB - 1}"][1])
    C.wait(dve, "dve", DMA, ev["ld_b2"][1])
    dve.append(("s2bias",))
    ev["s2bias"] = (V, C.inc(V, 1))

    # AG_p2 pieces (pool stream, after p1 gathers)
    for k in range(NPIECES):
        lastb = sum(pn[:k + 1]) - 1
        wk, wv = ev[f"p1_p2wr_b{lastb}"]
        C.wait(pool, "pool", wk, wv)
        pool.append(("ag", "p2", k))
        ev[f"ag_p2_{k}"] = (CC, C.inc(CC, 1))

    # ================= PHASE 2 =================
    emit_agg_phase("p2")

    # AG_h2 pieces
    for k in range(NPIECES):
        lastb = sum(pn[:k + 1]) - 1
        wk, wv = ev[f"p2_h2wr_b{lastb}"]
        C.wait(pool, "pool", wk, wv)
        pool.append(("ag", "h2", k))
        ev[f"ag_h2_{k}"] = (CC, C.inc(CC, 1))

    # ================= PHASE 3 =================
    # windows of <=8 groups within each gather segment
    wseq = [0]
    pending_expand = []

    def p3_emit_window(w, gw0, nb, slot, g0s):
        # pool: one-hot builds (unweighted)
        for g in range(gw0, gw0 + nb):
            C.wait(pool, "pool", DMA, ev["ld_dloc"][1])
            C.wait(pool, "pool", P, ev[f"p2_agg_g{Gtot - 1}"][1])
            if g - OHS >= 0:
                C.wait(pool, "pool", P, ev[f"p3_tr_g{g - OHS}"][1])
            pool.append(("ohu", g))
            ev[f"p3_oh_g{g}"] = (PL, C.inc(PL, 1))
        # PE: transposes into tb bank (w%2)
        for wi, g in enumerate(range(gw0, gw0 + nb)):
            C.wait(pe, "pe", PL, ev[f"p3_oh_g{g}"][1])
            if w - 2 >= 0:
                C.wait(pe, "pe", A, ev[f"p3_ocp_w{w - 2}"][1])
            if w < 2:
                lb = last_user[w % CB]
                C.wait(pe, "pe", V, ev[f"p2_h2add_b{lb}"][1])
            pe.append(("p3tr", g, wi, w % 2))
            ev[f"p3_tr_g{g}"] = (P, C.inc(P, 1))
        ev[f"p3_trdone_w{w}"] = (P, C.cur(P))
        # ACT: batched copy
        C.wait(act, "act", P, ev[f"p3_trdone_w{w}"][1])
        if w - 2 >= 0:
            C.wait(act, "act", P, ev[f"p3_expdone_w{w - 2}"][1])
        act.append(("p3ocp", w, nb))
        ev[f"p3_ocp_w{w}"] = (A, C.inc(A, 1))
        # PE: expands (deferred one window for pipelining)
        pending_expand.append((w, gw0, nb, slot, g0s))
        if len(pending_expand) > 1:
            p3_emit_expand(*pending_expand.pop(0))

    def p3_emit_expand(w, gw0, nb, slot, g0s):
        for wi, g in enumerate(range(gw0, gw0 + nb)):
            C.wait(pe, "pe", A, ev[f"p3_ocp_w{w}"][1])
            if w - 2 >= 0:
                C.wait(pe, "pe", V, ev[f"p3_multdone_w{w - 2}"][1])
            if w < 2:
                lb = last_user[2 + (w % 2)]
                C.wait(pe, "pe", V, ev[f"p2_h2add_b{lb}"][1])
            pe.append(("p3exp", g, wi, w % 2, block_of_g[g]))
            ev[f"p3_exp_g{g}"] = (P, C.inc(P, 1))
        ev[f"p3_expdone_w{w}"] = (P, C.cur(P))
        # DVE: batched mult lo/hi + reduce
        C.wait(dve, "dve", P, ev[f"p3_expdone_w{w}"][1])
        _gs, _gv = ev[f"p3_gather_g{g0s}"]
        C.wait(dve, "dve", _gs, _gv)
        if w - 2 >= 0:
            C.wait(dve, "dve", V, ev[f"p3_reddone_w{w - 2}"][1])
        dve.append(("p3mult", w, gw0, nb, slot, g0s, 0))
        dve.append(("p3mult", w, gw0, nb, slot, g0s, 1))
        mv = C.inc(V, 2)
        ev[f"p3_multdone_w{w}"] = (V, mv)
        for g in range(gw0, gw0 + nb):
            ev[f"p3_multdone_g{g}"] = (V, mv)
        C.wait(dve, "dve", V, ev[f"p3_multdone_w{w}"][1])
        dve.append(("p3red", w, gw0, nb))
        ev[f"p3_reddone_w{w}"] = (V, C.inc(V, 1))

    for (ci, k, g0s, gsum) in gsegs:
        slot = emit_gather("p3", ci, k, g0s, gsum)
        g = g0s
        while g < g0s + gsum:
            nb = min(8, g0s + gsum - g)
            p3_emit_window(wseq[0], g, nb, slot, g0s)
            wseq[0] += 1
            g += nb
    while pending_expand:
        p3_emit_expand(*pending_expand.pop(0))

    # sigmoid + writeback
    lastw = wseq[0] - 1
    C.wait(act, "act", V, ev[f"p3_reddone_w{lastw}"][1])
    act.append(("sigmoid",))
    ev["sig"] = (A, C.inc(A, 1))
    C.wait(sp, "sp", A, ev["sig"][1])
    sp.append(("sxwr",))
    C.inc(DMA, 16)

    # ------------------------------------------------ emit to bass
    from contextlib import ExitStack
    _es = ExitStack()
    with _es:
        idx1_sb = _es.enter_context(nc.sbuf_tensor("idx1_sb", [128, S // 16], I16))
        idx2_sb = _es.enter_context(nc.sbuf_tensor("idx2_sb", [128, S // 16], I16))
        dloc_sb = _es.enter_context(nc.sbuf_tensor("dloc_sb", [128, Gtot], F32))
        val_sb = _es.enter_context(nc.sbuf_tensor("val_sb", [128, Gtot], F32))
        vale_sb = _es.enter_context(nc.sbuf_tensor("vale_sb", [128, Gtot], F32))
        valo_sb = _es.enter_context(nc.sbuf_tensor("valo_sb", [128, Gtot], F32))
        xlT_sb = _es.enter_context(nc.sbuf_tensor("xlT_sb", [128, NPc], BF16))
        w1p_sb = _es.enter_context(nc.sbuf_tensor("w1p_sb", [128, 128], BF16))
        w1s_sb = _es.enter_context(nc.sbuf_tensor("w1s_sb", [128, 128], BF16))
        w2p_sb = _es.enter_context(nc.sbuf_tensor("w2p_sb", [128, 64], BF16))
        w2s_sb = _es.enter_context(nc.sbuf_tensor("w2s_sb", [128, 64], BF16))
        b1_sb = _es.enter_context(nc.sbuf_tensor("b1_sb", [128, 1], F32))
        b2_sb = _es.enter_context(nc.sbuf_tensor("b2_sb", [128, 64], F32))
        iota_sb = _es.enter_context(nc.sbuf_tensor("iota_sb", [128, 128], BF16))
        ident_sb = _es.enter_context(nc.sbuf_tensor("ident_sb", [128, 128], BF16))
        oh_sb = _es.enter_context(nc.sbuf_tensor("oh_sb", [128, OHS, 128], BF16))
        gbuf = _es.enter_context(
            nc.sbuf_tensor("gbuf", [128, NSLOT, GH * 128], BF16))
        h1T_sb = _es.enter_context(nc.sbuf_tensor("h1T_sb", [128, NPc], BF16))
        aggT_sb = _es.enter_context(nc.sbuf_tensor("aggT_sb", [128, 2, 128], BF16))
        s2_sb = _es.enter_context(nc.sbuf_tensor("s2_sb", [128, NB, 64], F32))
        h2nm_sb = _es.enter_context(nc.sbuf_tensor("h2nm_sb", [128, NB, 64], BF16))
        p2nm_sb = _es.enter_context(nc.sbuf_tensor("p2nm_sb", [128, NB, 64], BF16))
        h2pre_sb = _es.enter_context(nc.sbuf_tensor("h2pre_sb", [128, 2, 64], F32))
        osb_sb = _es.enter_context(nc.sbuf_tensor("osb_sb", [128, 2, 8 * 128], BF16))
        prod_sb = _es.enter_context(nc.sbuf_tensor("prod_sb", [128, 2, 8, 64], F32))
        dots_sb = _es.enter_context(nc.sbuf_tensor("dots_sb", [128, Gtot], F32))
        aggb = [_es.enter_context(nc.psum_tensor(f"aggb{j}", [128, 512], F32))
                for j in range(CB)]
        h1b = _es.enter_context(nc.psum_tensor("h1b", [128, 512], F32))
        p2s2b = _es.enter_context(nc.psum_tensor("p2s2b", [128, 512], F32))
        dma_s = _es.enter_context(nc.semaphore("dma_s"))
        gt_sems = [_es.enter_context(nc.semaphore(f"gt{j}_s"))
                   for j in range(NSLOT)]
        wr_sems = [_es.enter_context(nc.semaphore(f"wr{j}_s"))
                   for j in range(NPIECES)]
        v_s = _es.enter_context(nc.semaphore("v_s"))
        a_s = _es.enter_context(nc.semaphore("a_s"))
        p_s = _es.enter_context(nc.semaphore("p_s"))
        pl_s = _es.enter_context(nc.semaphore("pl_s"))
        cc_s = _es.enter_context(nc.semaphore("cc_s"))
        block = _es.enter_context(nc.Block())
        sems = {DMA: dma_s, V: v_s, A: a_s, P: p_s, PL: pl_s, CC: cc_s}
        for j in range(NSLOT):
            sems[GTS[j]] = gt_sems[j]
        for j in range(NPIECES):
            sems[WRS[j]] = wr_sems[j]

        sb_map = {"idx1": idx1_sb, "idx2": idx2_sb, "dloc": dloc_sb,
                  "val": val_sb, "vale": vale_sb, "valo": valo_sb,
                  "xlT": xlT_sb, "w1p": w1p_sb, "w1s": w1s_sb,
                  "w2p": w2p_sb, "w2s": w2s_sb, "b1": b1_sb, "b2": b2_sb}
        in_map = {"idx1": idx1_in, "idx2": idx2_in, "dloc": dloc_in,
                  "val": val_in, "vale": vale_in, "valo": valo_in,
                  "xlT": xlT_in, "w1p": w1p_in, "w1s": w1s_in,
                  "w2p": w2p_in, "w2s": w2s_in, "b1": b1_in, "b2": b2_in}

        def gv(slot, j):
            return gbuf[:, slot, j * 128:(j + 1) * 128]

        def run_ops(eng, name):
            for op in ops[name]:
                kind = op[0]
                if kind == "wait":
                    eng.wait_ge(sems[op[1]], op[2])
                elif kind == "dma_sb":
                    eng.dma_start(out=sb_map[op[1]][:], in_=in_map[op[1]][:]
                                  ).then_inc(dma_s, 16)
                elif kind == "iota":
                    eng.iota(iota_sb[:], pattern=[[1, 128]], base=0,
                             channel_multiplier=0,
                             allow_small_or_imprecise_dtypes=True)
                    eng.drain()
                elif kind == "ident":
                    eng.memset(ident_sb[:], 0.0)
                    eng.drain()
                    masks.make_identity(nc, ident_sb[:], nomemset=True)
                    eng.drain()
                    eng.memset(ident_sb[:1, :1], 1.0).then_inc(pl_s, 1)
                elif kind == "gather":
                    _, ph, k, g0, gsum, slot = op
                    if ph == "p1":
                        tbl = xg[int(trowbase[k]):int(trowbase[k + 1]), :]
                        idxs = idx1_sb
                    else:
                        t = p2t if ph == "p2" else h2t
                        tbl = t[pairbase[k]:pairbase[k + 1], :]
                        idxs = idx2_sb
                    eng.dma_gather(
                        gbuf[:, slot, :gsum * 128].rearrange(
                            "p (g f) -> p g f", f=128),
                        tbl,
                        idxs[:, g0 * 8:(g0 + gsum) * 8],
                        num_idxs=gsum * 128, num_idxs_reg=gsum * 128,
                        elem_size=128, single_packet=False,
                    ).then_inc(gt_sems[slot], 16)
                elif kind == "ohw":
                    _, g, vname, sh = op
                    vsb = {"val": val_sb, "vale": vale_sb, "valo": valo_sb}[vname]
                    slot = g % OHS if vname == "val" else (2 * g + sh) % OHS
                    sem = pl_s if vname == "valo" else v_s
                    eng.tensor_scalar(out=oh_sb[:, slot, :],
                                      in0=iota_sb[:],
                                      scalar1=dloc_sb[:, g:g + 1],
                                      scalar2=vsb[:, g:g + 1],
                                      op0=ALU.is_equal,
                                      op1=ALU.mult).then_inc(sem, 1)
                elif kind == "ohu":
                    g = op[1]
                    eng.tensor_scalar(out=oh_sb[:, g % OHS, :],
                                      in0=iota_sb[:],
                                      scalar1=dloc_sb[:, g:g + 1],
                                      scalar2=None,
                                      op0=ALU.is_equal).then_inc(pl_s, 1)
                elif kind == "agg1":
                    _, b, g, j, slot, first, last = op
                    eng.matmul(aggb[b % CB][:, :128], lhsT=gv(slot, j),
                               rhs=oh_sb[:, g % OHS, :], start=first,
                               stop=last).then_inc(p_s, 1)
                elif kind == "agg2":
                    _, b, g, j, slot, first, last = op
                    gvj = gv(slot, j)
                    eng.matmul(aggb[b % CB][:, :64],
                               lhsT=oh_sb[:, (2 * g) % OHS, :],
                               rhs=gvj[:, :64], start=first,
                               stop=False).then_inc(p_s, 1)
                    eng.matmul(aggb[b % CB][:, :64],
                               lhsT=oh_sb[:, (2 * g + 1) % OHS, :],
                               rhs=gvj[:, 64:], start=False,
                               stop=last).then_inc(p_s, 1)
                elif kind == "aggcopy":
                    b = op[1]
                    eng.activation(aggT_sb[:, b % 2, :], aggb[b % CB][:, :128],
                                   AF.Copy).then_inc(a_s, 1)
                elif kind == "h1mm":
                    b = op[1]
                    eng.matmul(h1b[:, :128], lhsT=w1p_sb[:],
                               rhs=aggT_sb[:, b % 2, :], start=True,
                               stop=False).then_inc(p_s, 1)
                    eng.matmul(h1b[:, :128], lhsT=w1s_sb[:],
                               rhs=xlT_sb[:, b * 128:(b + 1) * 128],
                               start=False, stop=True).then_inc(p_s, 1)
                elif kind == "h1relu":
                    b = op[1]
                    eng.activation(h1T_sb[:, b * 128:(b + 1) * 128],
                                   h1b[:, :128], AF.Relu, bias=b1_sb[:]
                                   ).then_inc(a_s, 1)
                elif kind == "p2mm":
                    b = op[1]
                    eng.matmul(p2s2b[:, :64],
                               lhsT=h1T_sb[:, b * 128:(b + 1) * 128],
                               rhs=w2p_sb[:], start=True, stop=True
                               ).then_inc(p_s, 1)
                    eng.matmul(p2s2b[:, 64:128],
                               lhsT=h1T_sb[:, b * 128:(b + 1) * 128],
                               rhs=w2s_sb[:], start=True, stop=True
                               ).then_inc(p_s, 1)
                elif kind == "p2cp":
                    b = op[1]
                    eng.activation(p2nm_sb[:, b, :], p2s2b[:, :64],
                                   AF.Copy).then_inc(a_s, 1)
                elif kind == "s2cp":
                    b = op[1]
                    eng.activation(s2_sb[:, b, :], p2s2b[:, 64:128],
                                   AF.Copy).then_inc(a_s, 1)
                elif kind == "p2wr":
                    b = op[1]
                    eng.dma_start(out=p2_loc[b * 128:(b + 1) * 128, :],
                                  in_=p2nm_sb[:, b, :]).then_inc(
                        wr_sems[piece_of_block[b]], 16)
                elif kind == "s2bias":
                    eng.tensor_tensor(
                        out=s2_sb[:], in0=s2_sb[:],
                        in1=b2_sb[:, None, :].to_broadcast([128, NB, 64]),
                        op=ALU.add).then_inc(v_s, 1)
                elif kind == "ag":
                    _, which, k = op
                    loc = p2_loc if which == "p2" else h2_loc
                    tab = p2t if which == "p2" else h2t
                    eng.collective_compute(
                        "AllGather", ALU.bypass,
                        replica_groups=[list(range(NCORES))],
                        ins=[loc[locbase[k]:locbase[k] + pn[k] * 128, :]],
                        outs=[tab[pairbase[k]:pairbase[k + 1], :]],
                    ).then_inc(cc_s, 1)
                elif kind == "h2add":
                    b = op[1]
                    eng.tensor_tensor(out=h2pre_sb[:, b % 2, :],
                                      in0=aggb[b % CB][:, :64],
                                      in1=s2_sb[:, b, :],
                                      op=ALU.add).then_inc(v_s, 1)
                elif kind == "h2relu":
                    b = op[1]
                    eng.activation(h2nm_sb[:, b, :], h2pre_sb[:, b % 2, :],
                                   AF.Relu).then_inc(a_s, 1)
                elif kind == "h2wr":
                    b = op[1]
                    eng.dma_start(out=h2_loc[b * 128:(b + 1) * 128, :],
                                  in_=h2nm_sb[:, b, :]).then_inc(
                        wr_sems[piece_of_block[b]], 16)
                elif kind == "p3tr":
                    _, g, wi, tb = op
                    eng.transpose(
                        out=aggb[tb][:].bitcast(BF16)[:, wi * 128:(wi + 1) * 128],
                        in_=oh_sb[:, g % OHS, :],
                        identity=ident_sb[:]).then_inc(p_s, 1)
                elif kind == "p3ocp":
                    _, w, nb = op
                    eng.activation(
                        osb_sb[:, w % 2, :nb * 128],
                        aggb[w % 2][:].bitcast(BF16)[:, :nb * 128],
                        AF.Copy).then_inc(a_s, 1)
                elif kind == "p3exp":
                    _, g, wi, eb, b = op
                    eng.matmul(aggb[2 + eb][:, wi * 64:(wi + 1) * 64],
                               lhsT=osb_sb[:, eb, wi * 128:(wi + 1) * 128],
                               rhs=h2nm_sb[:, b, :], start=True, stop=True
                               ).then_inc(p_s, 1)
                elif kind == "p3mult":
                    _, w, gw0, nb, slot, g0s, hi = op
                    pr = slice(64, 128) if hi else slice(0, 64)
                    cr = slice(64, 128) if hi else slice(0, 64)
                    j0 = gw0 - g0s
                    eng.tensor_tensor(
                        out=prod_sb[pr, w % 2, :nb, :],
                        in0=aggb[2 + w % 2][pr, :nb * 64].rearrange(
                            "p (g f) -> p g f", f=64),
                        in1=gbuf[pr, slot, j0 * 128:(j0 + nb) * 128].rearrange(
                            "p (g f) -> p g f", f=128)[:, :, cr],
                        op=ALU.mult).then_inc(v_s, 1)
                elif kind == "p3red":
                    _, w, gw0, nb = op
                    eng.reduce_sum(out=dots_sb[:, gw0:gw0 + nb],
                                   in_=prod_sb[:, w % 2, :nb, :],
                                   axis=mybir.AxisListType.X).then_inc(v_s, 1)
                elif kind == "sigmoid":
                    eng.activation(dots_sb[:], dots_sb[:], AF.Sigmoid
                                   ).then_inc(a_s, 1)
                elif kind == "sxwr":
                    eng.dma_start(out=sx_out[:], in_=dots_sb[:]
                                  ).then_inc(dma_s, 16)
                else:
                    raise ValueError(kind)

        @block.sync
        def _(e):
            run_ops(e, "sp")

        @block.gpsimd
        def _(e):
            run_ops(e, "pool")

        @block.vector
        def _(e):
            run_ops(e, "dve")

        @block.scalar
        def _(e):
            run_ops(e, "act")

        @block.tensor
        def _(e):
            run_ops(e, "pe")

    nc.compile()
    return nc


# ---------------------------------------------------------------- host glue
def host_prep(X, edge_row, edge_col, edge_vals, W1p, b1p, W1s, b1s,
              W2p, b2p, W2s, b2s, plan):
    p = plan
    NP, NPc = p.NP, p.NPc
    Xp = np.zeros((NP, X.shape[1]), np.float32)
    Xp[: X.shape[0]] = X
    Xperm = Xp[p.perm]                       # (c,b,l)-ordered
    Xtab = Xp[p.node_of_trow]                # table-row ordered
    Xg = Xtab.astype(ml_dtypes.bfloat16)
    b1 = np.ascontiguousarray((b1p + b1s).astype(np.float32)[:, None])
    b2rep = np.ascontiguousarray(
        np.tile((b2p + b2s).astype(np.float32)[None, :], (128, 1)))
    in_maps = []
    for c in range(NCORES):
        in_maps.append({
            "xg": Xg,
            "xlT": np.ascontiguousarray(
                Xperm[c * NPc:(c + 1) * NPc].T).astype(ml_dtypes.bfloat16),
            "idx1": wrap_idx(p.idx1[c]),
            "idx2": wrap_idx(p.idx2[c]),
            "dloc": colmajor(p.dloc[c]),
            "val": colmajor(p.val[c]),
            "vale": colmajor(p.val[c]) * p.parmask[:, None],
            "valo": colmajor(p.val[c]) * (1.0 - p.parmask)[:, None],
            "w1p": np.ascontiguousarray(W1p).astype(ml_dtypes.bfloat16),
            "w1s": np.ascontiguousarray(W1s).astype(ml_dtypes.bfloat16),
            "w2p": np.ascontiguousarray(W2p).astype(ml_dtypes.bfloat16),
            "w2s": np.ascontiguousarray(W2s).astype(ml_dtypes.bfloat16),
            "b1": b1, "b2rep": b2rep,
        })
    return in_maps


def unpermute_sx(results, plan, n_edges):
    p = plan
    sx = np.empty(n_edges, np.float32)
    for c in range(NCORES):
        flat = results[c]["sx"].T.reshape(-1)
        m = p.core_of_edge[:n_edges] == c
        sx[m] = flat[p.slot_of_edge[m]]
    return sx


_CACHE = {}


def kernel(X, edge_row, edge_col, edge_vals,
           W_pass1, b_pass1, W_self1, b_self1,
           W_pass2, b_pass2, W_self2, b_self2):
    X = np.asarray(X, np.float32)
    er = np.asarray(edge_row).astype(np.int64)
    ec = np.asarray(edge_col).astype(np.int64)
    ev_ = np.asarray(edge_vals, np.float32)
    n_nodes, n_edges = X.shape[0], len(er)

    key = (n_nodes, n_edges, int(er[0]), int(ec[0]))
    if key not in _CACHE:
        plan = plan_graph(er, ec, ev_, n_nodes)
        nc = build(plan)
        _CACHE[key] = (plan, nc)
    plan, nc = _CACHE[key]

    in_maps = host_prep(X, er, ec, ev_,
                        np.asarray(W_pass1), np.asarray(b_pass1),
                        np.asarray(W_self1), np.asarray(b_self1),
                        np.asarray(W_pass2), np.asarray(b_pass2),
                        np.asarray(W_self2), np.asarray(b_self2), plan)
    res = run_bass_kernel_spmd(nc, in_maps, core_ids=list(range(NCORES)))
    return unpermute_sx(res.results, plan, n_edges)


# revision 15
# speedup vs baseline: 1.0151x; 1.0151x over previous
"""Self-contained GCN edge-dot kernel for 8 TRN2 NeuronCores (v2).

kernel(**inputs) takes the FULL problem inputs and returns sigmoid edge
scores for every edge, computed SPMD across 8 cores with bass/bacc.

Strategy vs v1: nodes assigned degree-balanced to (core, block) with a
piece-major gather-table numbering so the two inter-layer exchanges
(P2 = H1 @ W_pass2 and H2, both 64-wide bf16 packed two-nodes-per-256B
row) stream as per-piece AllGather collectives overlapped with compute;
one-hot selection matrices built per group on DVE/Pool via
tensor_scalar (is_equal x val, 4x DVE mode); phase-3 edge dot uses
PE transpose + batched ACT psum copy + batched DVE mult/reduce; edge
val folded into the one-hot so gathered rows are used unscaled.
"""
import sys
sys.path.insert(0, "/opt/trn_rl_repo")
import numpy as np
import ml_dtypes
import concourse.bass as bass
import concourse.bacc as bacc
import concourse.mybir as mybir
from concourse import masks
from concourse.bass_utils import run_bass_kernel_spmd

F32 = mybir.dt.float32
BF16 = mybir.dt.bfloat16
I16 = mybir.dt.int16
AF = mybir.ActivationFunctionType
ALU = mybir.AluOpType
NCORES = 8


# ---------------------------------------------------------------- host planning
class Plan:
    pass


def plan_graph(edge_row, edge_col, edge_vals, n_nodes, NB=49, CB=4,
               pstart=(0, 25)):
    p = Plan()
    NPc = NB * 128
    NP = NPc * NCORES
    assert n_nodes <= NP
    NPIECES = len(pstart)
    pstart = list(pstart) + [NB]
    pn = [pstart[k + 1] - pstart[k] for k in range(NPIECES)]
    p.NB, p.NPc, p.NP, p.CB, p.NPIECES = NB, NPc, NP, CB, NPIECES
    p.pn = pn

    E = len(edge_row)
    deg = np.bincount(edge_row, minlength=NP)
    order = np.argsort(-deg, kind="stable")
    nblocks = NCORES * NB
    newpos = np.empty(NP, np.int64)
    for g in range(nblocks):
        members = order[g::nblocks]
        c, b = g // NB, g % NB
        newpos[members] = c * NPc + b * 128 + np.arange(len(members))
    p.newpos = newpos
    perm = np.empty(NP, np.int64)
    perm[newpos] = np.arange(NP)
    p.perm = perm

    # piece-major gather-table row numbering: trow(c,b,l) for piece k
    #   = trowbase[k] + c*pn[k]*128 + (b - pstart[k])*128 + l
    trowbase = np.zeros(NPIECES + 1, np.int64)
    for k in range(NPIECES):
        trowbase[k + 1] = trowbase[k] + NCORES * pn[k] * 128
    p.trowbase = trowbase
    pos_c = newpos // NPc
    pos_b = (newpos % NPc) // 128
    pos_l = newpos % 128
    pn_arr = np.array(pn)
    pstart_arr = np.array(pstart[:NPIECES])
    pk = np.searchsorted(pstart_arr[1:] if NPIECES > 1 else np.array([NB]),
                         pos_b, side="right")
    pk = np.minimum(pk, NPIECES - 1)
    trow_of_node = (trowbase[pk] + pos_c * pn_arr[pk] * 128 +
                    (pos_b - pstart_arr[pk]) * 128 + pos_l)
    p.trow_of_node = trow_of_node  # original node id -> table row
    # inverse: table row -> original node id
    node_of_trow = np.empty(NP, np.int64)
    node_of_trow[trow_of_node] = np.arange(NP)
    p.node_of_trow = node_of_trow

    nr = newpos[edge_row]          # dest in (c,b,l) space
    core = nr // NPc
    blk = (nr % NPc) // 128
    dloc = nr % 128
    strow = trow_of_node[edge_col]  # source table row
    spiece = np.searchsorted(trowbase[1:NPIECES + 1], strow, side="right")
    sidx1 = strow - trowbase[spiece]            # idx within piece (p1, rows)
    sidx2 = sidx1 // 2                          # idx within piece (p2/p3 pairs)
    par = (strow % 2).astype(np.int64)          # parity within pair row

    # bucket (core, block, piece) -> even/odd edge lists
    G = np.zeros((NB, NPIECES), np.int64)
    buckets = {}
    for c in range(NCORES):
        m_c = core == c
        for b in range(NB):
            m_b = m_c & (blk == b)
            for k in range(NPIECES):
                m = m_b & (spiece == k)
                ev_ = np.nonzero(m & (par == 0))[0]
                od_ = np.nonzero(m & (par == 1))[0]
                buckets[(c, b, k)] = (ev_, od_)
                ng = max(-(-len(ev_) // 64), -(-len(od_) // 64))
                G[b, k] = max(G[b, k], ng)
    for b in range(NB):
        if G[b].sum() == 0:
            G[b, 0] = 1
    p.G = G
    p.Gtot = int(G.sum())
    S = p.Gtot * 128

    chunks = [list(range(i, min(i + CB, NB))) for i in range(0, NB, CB)]
    p.chunks = chunks
    segs = []
    gidx = 0
    firstg = {}
    lastg = {}
    for ci, cblocks in enumerate(chunks):
        for k in range(NPIECES):
            for b in cblocks:
                ng = int(G[b, k])
                if ng == 0:
                    continue
                segs.append((ci, k, b, gidx, ng))
                if b not in firstg:
                    firstg[b] = gidx
                lastg[b] = gidx + ng - 1
                gidx += ng
    assert gidx == p.Gtot
    p.segs = segs
    p.firstg, p.lastg = firstg, lastg
    # gather segments: one dma_gather per (ci, k) covering its blocks' groups
    gsegs = []
    i = 0
    while i < len(segs):
        ci, k, b, g0, ng = segs[i]
        j = i
        tot = 0
        while j < len(segs) and segs[j][0] == ci and segs[j][1] == k:
            tot += segs[j][4]
            j += 1
        gsegs.append((ci, k, g0, tot))
        i = j
    p.gsegs = gsegs
    p.GH = max(t for (_, _, _, t) in gsegs)
    p.block_of_g = {}
    for (ci, k, b, g0, ng) in segs:
        for g in range(g0, g0 + ng):
            p.block_of_g[g] = b

    p.idx1 = np.zeros((NCORES, S), np.int16)
    p.idx2 = np.zeros((NCORES, S), np.int16)
    p.dloc = np.zeros((NCORES, S), np.float32)
    p.val = np.zeros((NCORES, S), np.float32)
    p.parmask = (np.arange(128) < 64).astype(np.float32)
    p.slot_of_edge = np.full(E, -1, np.int64)
    p.core_of_edge = core
    for c in range(NCORES):
        for (ci, k, b, g0, ng) in segs:
            ev_, od_ = buckets[(c, b, k)]
            for half, eids in ((0, ev_), (1, od_)):
                n = len(eids)
                assert n <= ng * 64
                gi = np.arange(n) // 64
                sl = (g0 + gi) * 128 + half * 64 + np.arange(n) % 64
                p.idx1[c, sl] = sidx1[eids]
                p.idx2[c, sl] = sidx2[eids]
                p.dloc[c, sl] = dloc[eids]
                p.val[c, sl] = edge_vals[eids]
                p.slot_of_edge[eids] = sl
    return p


def wrap_idx(idx_flat):
    S = len(idx_flat)
    w = idx_flat.reshape(S // 16, 16).T
    return np.tile(w, (8, 1)).copy()


def colmajor(a):
    S = len(a)
    return a.reshape(S // 128, 128).T.copy()


# ---------------------------------------------------------------- emission fw
class Counters:
    def __init__(self):
        self.val = {}
        self.last = {}

    def inc(self, sem, by):
        self.val[sem] = self.val.get(sem, 0) + by
        return self.val[sem]

    def cur(self, sem):
        return self.val.get(sem, 0)

    def wait(self, eng_ops, eng_name, sem, v):
        if v <= 0:
            return
        key = (eng_name, sem)
        if self.last.get(key, -1) >= v:
            return
        self.last[key] = v
        eng_ops.append(("wait", sem, v))


DMA, V, A, P, PL, CC = "dma", "v", "a", "p", "pl", "cc"
GTS = ("gt0", "gt1", "gt2", "gt3")
WRS = ("wr0", "wr1")


def build(plan):
    p = plan
    NB, NPc, NP, CB = p.NB, p.NPc, p.NP, p.CB
    NPIECES, pn, G = p.NPIECES, p.pn, p.G
    Gtot, segs, gsegs, chunks = p.Gtot, p.segs, p.gsegs, p.chunks
    firstg, lastg, block_of_g = p.firstg, p.lastg, p.block_of_g
    S = Gtot * 128
    GH = p.GH
    NSLOT = 4      # gather buffer slots
    OHS = 16       # one-hot slots

    nc = bacc.Bacc()
    dp = nc.declare_dram_parameter
    xg = dp("xg", [NP, 128], BF16, isOutput=False)
    xlT_in = dp("xlT", [128, NPc], BF16, isOutput=False)
    idx1_in = dp("idx1", [128, S // 16], I16, isOutput=False)
    idx2_in = dp("idx2", [128, S // 16], I16, isOutput=False)
    dloc_in = dp("dloc", [128, Gtot], F32, isOutput=False)
    val_in = dp("val", [128, Gtot], F32, isOutput=False)
    vale_in = dp("vale", [128, Gtot], F32, isOutput=False)
    valo_in = dp("valo", [128, Gtot], F32, isOutput=False)
    w1p_in = dp("w1p", [128, 128], BF16, isOutput=False)
    w1s_in = dp("w1s", [128, 128], BF16, isOutput=False)
    w2p_in = dp("w2p", [128, 64], BF16, isOutput=False)
    w2s_in = dp("w2s", [128, 64], BF16, isOutput=False)
    b1_in = dp("b1", [128, 1], F32, isOutput=False)
    b2_in = dp("b2rep", [128, 64], F32, isOutput=False)
    sx_out = dp("sx", [128, Gtot], F32, isOutput=True)

    p2_loc = nc.dram_tensor("p2_loc", [NPc, 64], BF16)
    h2_loc = nc.dram_tensor("h2_loc", [NPc, 64], BF16)
    p2t = nc.dram_tensor("p2t", [NP // 2, 128], BF16, addr_space="Shared")
    h2t = nc.dram_tensor("h2t", [NP // 2, 128], BF16, addr_space="Shared")

    # piece boundaries in table spaces
    trowbase = p.trowbase
    pairbase = [int(trowbase[k] // 2) for k in range(NPIECES + 1)]
    locbase = []   # local p2_loc/h2_loc row range per piece
    acc = 0
    for k in range(NPIECES):
        locbase.append(acc)
        acc += pn[k] * 128

    ops = {e: [] for e in ("sp", "pool", "dve", "act", "pe")}
    C = Counters()
    ev = {}
    sp, pool, dve, act, pe = (ops[k] for k in ("sp", "pool", "dve", "act", "pe"))

    # ---- loads
    _ld_names = ("idx1", "idx2", "dloc", "val", "vale", "valo", "xlT",
                 "w1p", "w1s", "w2p", "w2s", "b1", "b2")
    for name in _ld_names:
        sp.append(("dma_sb", name))
        C.inc(DMA, 16)
    for name in _ld_names:
        ev["ld_" + name] = (DMA, C.cur(DMA))
    pool.append(("iota",))
    pool.append(("ident",))
    ev["p0_pool"] = (PL, C.inc(PL, 1))

    # last p1/p2 block using psum bank j (for bank-reuse waits)
    last_user = {}
    for b in range(NB):
        last_user[b % CB] = b
    pstart_list = [0]
    for k in range(NPIECES - 1):
        pstart_list.append(pstart_list[-1] + pn[k])
    piece_of_block = {}
    for b in range(NB):
        piece_of_block[b] = max(k for k in range(NPIECES)
                                if pstart_list[k] <= b)

    # ---------------- gather emission helper
    gseq = [0]

    def emit_gather(ph, ci, k, g0, gsum):
        s = gseq[0]
        slot = s % NSLOT
        # slot reuse: wait consumers of gather s-NSLOT
        prev = s - NSLOT
        if prev >= 0:
            pph, pg0, pgsum = gmeta[prev]
            if pph == "p3":
                C.wait(pool, "pool", V, ev[f"p3_multdone_g{pg0 + pgsum - 1}"][1])
            else:
                C.wait(pool, "pool", P, ev[f"{pph}_agg_g{pg0 + pgsum - 1}"][1])
        C.wait(pool, "pool", DMA, ev["ld_idx1" if ph == "p1" else "ld_idx2"][1])
        if ph == "p2":
            C.wait(pool, "pool", CC, k + 1)
        elif ph == "p3":
            C.wait(pool, "pool", CC, NPIECES + k + 1)
        pool.append(("gather", ph, k, g0, gsum, slot))
        gs = GTS[slot]
        ev[f"{ph}_gather_g{g0}"] = (gs, C.inc(gs, 16))
        gmeta.append((ph, g0, gsum))
        gseq[0] += 1
        return slot

    gmeta = []

    # ---------------- p1 / p2 aggregation phases
    def emit_agg_phase(ph):
        for (ci, k, g0s, gsum) in gsegs:
            slot = emit_gather(ph, ci, k, g0s, gsum)
            segs_here = [(b, g0, ng) for (ci2, k2, b, g0, ng) in segs
                         if ci2 == ci and k2 == k]
            for (b, g0, ng) in segs_here:
                for g in range(g0, g0 + ng):
                    # weighted one-hot build(s)
                    if ph == "p1":
                        C.wait(dve, "dve", DMA, ev["ld_val"][1])
                        C.wait(dve, "dve", PL, ev["p0_pool"][1])
                        if g - OHS >= 0:
                            C.wait(dve, "dve", P, ev[f"p1_agg_g{g - OHS}"][1])
                        dve.append(("ohw", g, "val", 0))
                        ev[f"p1_oh_g{g}"] = (V, C.inc(V, 1))
                    else:
                        # two parity-masked builds: even on DVE, odd on POOL
                        # p2 uses oh slots (2g)%OHS and (2g+1)%OHS
                        lue = (2 * g) % OHS + OHS * ((Gtot - 1 - (2 * g) % OHS) // OHS)
                        luo = (2 * g + 1) % OHS + OHS * ((Gtot - 1 - (2 * g + 1) % OHS) // OHS)
                        C.wait(dve, "dve", DMA, ev["ld_vale"][1])
                        C.wait(dve, "dve", P, ev[f"p1_agg_g{lue}"][1])
                        if g - OHS // 2 >= 0:
                            C.wait(dve, "dve", P, ev[f"p2_agg_g{g - OHS // 2}"][1])
                        dve.append(("ohw", g, "vale", 0))
                        ev[f"p2_ohe_g{g}"] = (V, C.inc(V, 1))
                        C.wait(pool, "pool", DMA, ev["ld_valo"][1])
                        C.wait(pool, "pool", P, ev[f"p1_agg_g{luo}"][1])
                        if g - OHS // 2 >= 0:
                            C.wait(pool, "pool", P, ev[f"p2_agg_g{g - OHS // 2}"][1])
                        pool.append(("ohw", g, "valo", 1))
                        ev[f"p2_oho_g{g}"] = (PL, C.inc(PL, 1))

                    # PE: aggregation matmul(s)
                    _gs, _gv = ev[f"{ph}_gather_g{g0s}"]
                    C.wait(pe, "pe", _gs, _gv)
                    if ph == "p1":
                        C.wait(pe, "pe", V, ev[f"p1_oh_g{g}"][1])
                    else:
                        C.wait(pe, "pe", V, ev[f"p2_ohe_g{g}"][1])
                        C.wait(pe, "pe", PL, ev[f"p2_oho_g{g}"][1])
                    first = g == firstg[b]
                    last = g == lastg[b]
                    if first:
                        # psum bank reuse
                        if ph == "p1":
                            if b - CB >= 0:
                                C.wait(pe, "pe", A, ev[f"p1_aggcopy_b{b - CB}"][1])
                        else:
                            lb = last_user[b % CB]
                            C.wait(pe, "pe", A, ev[f"p1_aggcopy_b{lb}"][1])
                            if b - CB >= 0:
                                C.wait(pe, "pe", V, ev[f"p2_h2add_b{b - CB}"][1])
                    j = g - g0s
                    if ph == "p1":
                        pe.append(("agg1", b, g, j, slot, first, last))
                        ev[f"p1_agg_g{g}"] = (P, C.inc(P, 1))
                    else:
                        pe.append(("agg2", b, g, j, slot, first, last))
                        ev[f"p2_agg_g{g}"] = (P, C.inc(P, 2))
                    if last:
                        ev[f"{ph}_aggstop_b{b}"] = (P, C.cur(P))
                        if ph == "p1":
                            p1_block_tail(b)
                        else:
                            p2_block_tail(b)

    def p1_block_tail(b):
        # ACT: aggcopy psum->sbuf bf16
        C.wait(act, "act", P, ev[f"p1_aggstop_b{b}"][1])
        if b - 2 >= 0:
            C.wait(act, "act", P, ev[f"p1_h1mm_b{b - 2}"][1])
        act.append(("aggcopy", b))
        ev[f"p1_aggcopy_b{b}"] = (A, C.inc(A, 1))
        # PE: h1 = relu(W1p^T aggT + W1s^T xlT + b1)
        C.wait(pe, "pe", DMA, ev["ld_xlT"][1])
        C.wait(pe, "pe", A, ev[f"p1_aggcopy_b{b}"][1])
        if b - 1 >= 0:
            C.wait(pe, "pe", A, ev[f"p1_h1relu_b{b - 1}"][1])
        pe.append(("h1mm", b))
        ev[f"p1_h1mm_b{b}"] = (P, C.inc(P, 2))
        C.wait(act, "act", P, ev[f"p1_h1mm_b{b}"][1])
        C.wait(act, "act", DMA, ev["ld_b1"][1])
        act.append(("h1relu", b))
        ev[f"p1_h1relu_b{b}"] = (A, C.inc(A, 1))
        # PE: P2 block + S2 block
        C.wait(pe, "pe", DMA, ev["ld_w2s"][1])
        C.wait(pe, "pe", A, ev[f"p1_h1relu_b{b}"][1])
        if b - 1 >= 0:
            C.wait(pe, "pe", A, ev[f"p1_s2cp_b{b - 1}"][1])
        pe.append(("p2mm", b))
        ev[f"p1_p2mm_b{b}"] = (P, C.inc(P, 2))
        C.wait(act, "act", P, ev[f"p1_p2mm_b{b}"][1])
        act.append(("p2cp", b))
        ev[f"p1_p2cp_b{b}"] = (A, C.inc(A, 1))
        act.append(("s2cp", b))
        ev[f"p1_s2cp_b{b}"] = (A, C.inc(A, 1))
        C.wait(sp, "sp", A, ev[f"p1_p2cp_b{b}"][1])
        wk = WRS[piece_of_block[b]]
        C.wait(sp, "sp", wk, C.cur(wk))
        sp.append(("p2wr", b))
        ev[f"p1_p2wr_b{b}"] = (wk, C.inc(wk, 16))

    def p2_block_tail(b):
        C.wait(dve, "dve", P, ev[f"p2_aggstop_b{b}"][1])
        C.wait(dve, "dve", V, ev["s2bias"][1])
        if b - 2 >= 0:
            C.wait(dve, "dve", A, ev[f"p2_h2relu_b{b - 2}"][1])
        dve.append(("h2add", b))
        ev[f"p2_h2add_b{b}"] = (V, C.inc(V, 1))
        C.wait(act, "act", V, ev[f"p2_h2add_b{b}"][1])
        act.append(("h2relu", b))
        ev[f"p2_h2relu_b{b}"] = (A, C.inc(A, 1))
        C.wait(sp, "sp", A, ev[f"p2_h2relu_b{b}"][1])
        wk = WRS[piece_of_block[b]]
        C.wait(sp, "sp", wk, C.cur(wk))
        sp.append(("h2wr", b))
        ev[f"p2_h2wr_b{b}"] = (wk, C.inc(wk, 16))

    # ================= PHASE 1 =================
    emit_agg_phase("p1")

    # s2bias after all s2cp
    C.wait(dve, "dve", A, ev[f"p1_s2cp_b{NB - 1}"][1])
    C.wait(dve, "dve", DMA, ev["ld_b2"][1])
    dve.append(("s2bias",))
    ev["s2bias"] = (V, C.inc(V, 1))

    # AG_p2 pieces (pool stream, after p1 gathers)
    for k in range(NPIECES):
        lastb = sum(pn[:k + 1]) - 1
        wk, wv = ev[f"p1_p2wr_b{lastb}"]
        C.wait(pool, "pool", wk, wv)
        pool.append(("ag", "p2", k))
        ev[f"ag_p2_{k}"] = (CC, C.inc(CC, 1))

    # ================= PHASE 2 =================
    emit_agg_phase("p2")

    # AG_h2 pieces
    for k in range(NPIECES):
        lastb = sum(pn[:k + 1]) - 1
        wk, wv = ev[f"p2_h2wr_b{lastb}"]
        C.wait(pool, "pool", wk, wv)
        pool.append(("ag", "h2", k))
        ev[f"ag_h2_{k}"] = (CC, C.inc(CC, 1))

    # ================= PHASE 3 =================
    # windows of <=8 groups within each gather segment
    wseq = [0]
    pending_expand = []

    def p3_emit_window(w, gw0, nb, slot, g0s):
        # pool: one-hot builds (unweighted)
        for g in range(gw0, gw0 + nb):
            C.wait(pool, "pool", DMA, ev["ld_dloc"][1])
            C.wait(pool, "pool", P, ev[f"p2_agg_g{Gtot - 1}"][1])
            if g - OHS >= 0:
                C.wait(pool, "pool", P, ev[f"p3_tr_g{g - OHS}"][1])
            pool.append(("ohu", g))
            ev[f"p3_oh_g{g}"] = (PL, C.inc(PL, 1))
        # PE: transposes into tb bank (w%2)
        for wi, g in enumerate(range(gw0, gw0 + nb)):
            C.wait(pe, "pe", PL, ev[f"p3_oh_g{g}"][1])
            if w - 2 >= 0:
                C.wait(pe, "pe", A, ev[f"p3_ocp_w{w - 2}"][1])
            if w < 2:
                C.wait(pe, "pe", V, ev[f"p2_h2add_b{NB - 1}"][1])
            pe.append(("p3tr", g, wi, w % 2))
            ev[f"p3_tr_g{g}"] = (P, C.inc(P, 1))
        ev[f"p3_trdone_w{w}"] = (P, C.cur(P))
        # ACT: batched copy
        C.wait(act, "act", P, ev[f"p3_trdone_w{w}"][1])
        if w - 2 >= 0:
            C.wait(act, "act", P, ev[f"p3_expdone_w{w - 2}"][1])
        act.append(("p3ocp", w, nb))
        ev[f"p3_ocp_w{w}"] = (A, C.inc(A, 1))
        # PE: expands (deferred one window for pipelining)
        pending_expand.append((w, gw0, nb, slot, g0s))
        if len(pending_expand) > 1:
            p3_emit_expand(*pending_expand.pop(0))

    def p3_emit_expand(w, gw0, nb, slot, g0s):
        for wi, g in enumerate(range(gw0, gw0 + nb)):
            C.wait(pe, "pe", A, ev[f"p3_ocp_w{w}"][1])
            if w - 2 >= 0:
                C.wait(pe, "pe", V, ev[f"p3_multdone_w{w - 2}"][1])
            if w < 2:
                C.wait(pe, "pe", A, ev[f"p1_h1relu_b{NB - 1}"][1])
                C.wait(pe, "pe", A, ev[f"p1_s2cp_b{NB - 1}"][1])
            pe.append(("p3exp", g, wi, w % 2, block_of_g[g]))
            ev[f"p3_exp_g{g}"] = (P, C.inc(P, 1))
        ev[f"p3_expdone_w{w}"] = (P, C.cur(P))
        # DVE: batched mult lo/hi + reduce
        C.wait(dve, "dve", P, ev[f"p3_expdone_w{w}"][1])
        _gs, _gv = ev[f"p3_gather_g{g0s}"]
        C.wait(dve, "dve", _gs, _gv)
        if w - 2 >= 0:
            C.wait(dve, "dve", V, ev[f"p3_reddone_w{w - 2}"][1])
        dve.append(("p3mult", w, gw0, nb, slot, g0s, 0))
        dve.append(("p3mult", w, gw0, nb, slot, g0s, 1))
        mv = C.inc(V, 2)
        ev[f"p3_multdone_w{w}"] = (V, mv)
        for g in range(gw0, gw0 + nb):
            ev[f"p3_multdone_g{g}"] = (V, mv)
        C.wait(dve, "dve", V, ev[f"p3_multdone_w{w}"][1])
        dve.append(("p3red", w, gw0, nb))
        ev[f"p3_reddone_w{w}"] = (V, C.inc(V, 1))

    for (ci, k, g0s, gsum) in gsegs:
        slot = emit_gather("p3", ci, k, g0s, gsum)
        g = g0s
        while g < g0s + gsum:
            nb = min(16, g0s + gsum - g)
            p3_emit_window(wseq[0], g, nb, slot, g0s)
            wseq[0] += 1
            g += nb
    while pending_expand:
        p3_emit_expand(*pending_expand.pop(0))

    # sigmoid + writeback
    lastw = wseq[0] - 1
    C.wait(act, "act", V, ev[f"p3_reddone_w{lastw}"][1])
    act.append(("sigmoid",))
    ev["sig"] = (A, C.inc(A, 1))
    C.wait(sp, "sp", A, ev["sig"][1])
    sp.append(("sxwr",))
    C.inc(DMA, 16)

    # ------------------------------------------------ emit to bass
    from contextlib import ExitStack
    _es = ExitStack()
    with _es:
        idx1_sb = _es.enter_context(nc.sbuf_tensor("idx1_sb", [128, S // 16], I16))
        idx2_sb = _es.enter_context(nc.sbuf_tensor("idx2_sb", [128, S // 16], I16))
        dloc_sb = _es.enter_context(nc.sbuf_tensor("dloc_sb", [128, Gtot], F32))
        val_sb = _es.enter_context(nc.sbuf_tensor("val_sb", [128, Gtot], F32))
        vale_sb = _es.enter_context(nc.sbuf_tensor("vale_sb", [128, Gtot], F32))
        valo_sb = _es.enter_context(nc.sbuf_tensor("valo_sb", [128, Gtot], F32))
        xlT_sb = _es.enter_context(nc.sbuf_tensor("xlT_sb", [128, NPc], BF16))
        w1p_sb = _es.enter_context(nc.sbuf_tensor("w1p_sb", [128, 128], BF16))
        w1s_sb = _es.enter_context(nc.sbuf_tensor("w1s_sb", [128, 128], BF16))
        w2p_sb = _es.enter_context(nc.sbuf_tensor("w2p_sb", [128, 64], BF16))
        w2s_sb = _es.enter_context(nc.sbuf_tensor("w2s_sb", [128, 64], BF16))
        b1_sb = _es.enter_context(nc.sbuf_tensor("b1_sb", [128, 1], F32))
        b2_sb = _es.enter_context(nc.sbuf_tensor("b2_sb", [128, 64], F32))
        iota_sb = _es.enter_context(nc.sbuf_tensor("iota_sb", [128, 128], BF16))
        ident_sb = _es.enter_context(nc.sbuf_tensor("ident_sb", [128, 128], BF16))
        oh_sb = _es.enter_context(nc.sbuf_tensor("oh_sb", [128, OHS, 128], BF16))
        gbuf = _es.enter_context(
            nc.sbuf_tensor("gbuf", [128, NSLOT, GH * 128], BF16))
        h1T_sb = _es.enter_context(nc.sbuf_tensor("h1T_sb", [128, NPc], BF16))
        aggT_sb = _es.enter_context(nc.sbuf_tensor("aggT_sb", [128, 2, 128], BF16))
        s2_sb = _es.enter_context(nc.sbuf_tensor("s2_sb", [128, NB, 64], F32))
        h2nm_sb = _es.enter_context(nc.sbuf_tensor("h2nm_sb", [128, NB, 64], BF16))
        p2nm_sb = _es.enter_context(nc.sbuf_tensor("p2nm_sb", [128, NB, 64], BF16))
        h2pre_sb = _es.enter_context(nc.sbuf_tensor("h2pre_sb", [128, 2, 64], F32))
        osb_sb = _es.enter_context(nc.sbuf_tensor("osb_sb", [128, 2, 16 * 128], BF16))
        prod_sb = _es.enter_context(nc.sbuf_tensor("prod_sb", [128, 2, 16, 64], F32))
        dots_sb = _es.enter_context(nc.sbuf_tensor("dots_sb", [128, Gtot], F32))
        aggps = [_es.enter_context(nc.psum_tensor(f"aggps{j}", [128, 1024], F32))
                 for j in range(2)]
        h1b = _es.enter_context(nc.psum_tensor("h1b", [128, 1024], F32))
        p2s2b = _es.enter_context(nc.psum_tensor("p2s2b", [128, 1024], F32))

        def aggreg(b, w):
            t = aggps[(b % 4) // 2]
            c0 = ((b % 4) % 2) * 512
            return t[:, c0:c0 + w]
        dma_s = _es.enter_context(nc.semaphore("dma_s"))
        gt_sems = [_es.enter_context(nc.semaphore(f"gt{j}_s"))
                   for j in range(NSLOT)]
        wr_sems = [_es.enter_context(nc.semaphore(f"wr{j}_s"))
                   for j in range(NPIECES)]
        v_s = _es.enter_context(nc.semaphore("v_s"))
        a_s = _es.enter_context(nc.semaphore("a_s"))
        p_s = _es.enter_context(nc.semaphore("p_s"))
        pl_s = _es.enter_context(nc.semaphore("pl_s"))
        cc_s = _es.enter_context(nc.semaphore("cc_s"))
        block = _es.enter_context(nc.Block())
        sems = {DMA: dma_s, V: v_s, A: a_s, P: p_s, PL: pl_s, CC: cc_s}
        for j in range(NSLOT):
            sems[GTS[j]] = gt_sems[j]
        for j in range(NPIECES):
            sems[WRS[j]] = wr_sems[j]

        sb_map = {"idx1": idx1_sb, "idx2": idx2_sb, "dloc": dloc_sb,
                  "val": val_sb, "vale": vale_sb, "valo": valo_sb,
                  "xlT": xlT_sb, "w1p": w1p_sb, "w1s": w1s_sb,
                  "w2p": w2p_sb, "w2s": w2s_sb, "b1": b1_sb, "b2": b2_sb}
        in_map = {"idx1": idx1_in, "idx2": idx2_in, "dloc": dloc_in,
                  "val": val_in, "vale": vale_in, "valo": valo_in,
                  "xlT": xlT_in, "w1p": w1p_in, "w1s": w1s_in,
                  "w2p": w2p_in, "w2s": w2s_in, "b1": b1_in, "b2": b2_in}

        def gv(slot, j):
            return gbuf[:, slot, j * 128:(j + 1) * 128]

        def run_ops(eng, name):
            for op in ops[name]:
                kind = op[0]
                if kind == "wait":
                    eng.wait_ge(sems[op[1]], op[2])
                elif kind == "dma_sb":
                    eng.dma_start(out=sb_map[op[1]][:], in_=in_map[op[1]][:]
                                  ).then_inc(dma_s, 16)
                elif kind == "iota":
                    eng.iota(iota_sb[:], pattern=[[1, 128]], base=0,
                             channel_multiplier=0,
                             allow_small_or_imprecise_dtypes=True)
                    eng.drain()
                elif kind == "ident":
                    eng.memset(ident_sb[:], 0.0)
                    eng.drain()
                    masks.make_identity(nc, ident_sb[:], nomemset=True)
                    eng.drain()
                    eng.memset(ident_sb[:1, :1], 1.0).then_inc(pl_s, 1)
                elif kind == "gather":
                    _, ph, k, g0, gsum, slot = op
                    if ph == "p1":
                        tbl = xg[int(trowbase[k]):int(trowbase[k + 1]), :]
                        idxs = idx1_sb
                    else:
                        t = p2t if ph == "p2" else h2t
                        tbl = t[pairbase[k]:pairbase[k + 1], :]
                        idxs = idx2_sb
                    eng.dma_gather(
                        gbuf[:, slot, :gsum * 128].rearrange(
                            "p (g f) -> p g f", f=128),
                        tbl,
                        idxs[:, g0 * 8:(g0 + gsum) * 8],
                        num_idxs=gsum * 128, num_idxs_reg=gsum * 128,
                        elem_size=128, single_packet=False,
                    ).then_inc(gt_sems[slot], 16)
                elif kind == "ohw":
                    _, g, vname, sh = op
                    vsb = {"val": val_sb, "vale": vale_sb, "valo": valo_sb}[vname]
                    slot = g % OHS if vname == "val" else (2 * g + sh) % OHS
                    sem = pl_s if vname == "valo" else v_s
                    eng.tensor_scalar(out=oh_sb[:, slot, :],
                                      in0=iota_sb[:],
                                      scalar1=dloc_sb[:, g:g + 1],
                                      scalar2=vsb[:, g:g + 1],
                                      op0=ALU.is_equal,
                                      op1=ALU.mult).then_inc(sem, 1)
                elif kind == "ohu":
                    g = op[1]
                    eng.tensor_scalar(out=oh_sb[:, g % OHS, :],
                                      in0=iota_sb[:],
                                      scalar1=dloc_sb[:, g:g + 1],
                                      scalar2=None,
                                      op0=ALU.is_equal).then_inc(pl_s, 1)
                elif kind == "agg1":
                    _, b, g, j, slot, first, last = op
                    eng.matmul(aggreg(b, 128), lhsT=gv(slot, j),
                               rhs=oh_sb[:, g % OHS, :], start=first,
                               stop=last).then_inc(p_s, 1)
                elif kind == "agg2":
                    _, b, g, j, slot, first, last = op
                    gvj = gv(slot, j)
                    eng.matmul(aggreg(b, 64),
                               lhsT=oh_sb[:, (2 * g) % OHS, :],
                               rhs=gvj[:, :64], start=first,
                               stop=False).then_inc(p_s, 1)
                    eng.matmul(aggreg(b, 64),
                               lhsT=oh_sb[:, (2 * g + 1) % OHS, :],
                               rhs=gvj[:, 64:], start=False,
                               stop=last).then_inc(p_s, 1)
                elif kind == "aggcopy":
                    b = op[1]
                    eng.activation(aggT_sb[:, b % 2, :], aggreg(b, 128),
                                   AF.Copy).then_inc(a_s, 1)
                elif kind == "h1mm":
                    b = op[1]
                    eng.matmul(h1b[:, :128], lhsT=w1p_sb[:],
                               rhs=aggT_sb[:, b % 2, :], start=True,
                               stop=False).then_inc(p_s, 1)
                    eng.matmul(h1b[:, :128], lhsT=w1s_sb[:],
                               rhs=xlT_sb[:, b * 128:(b + 1) * 128],
                               start=False, stop=True).then_inc(p_s, 1)
                elif kind == "h1relu":
                    b = op[1]
                    eng.activation(h1T_sb[:, b * 128:(b + 1) * 128],
                                   h1b[:, :128], AF.Relu, bias=b1_sb[:]
                                   ).then_inc(a_s, 1)
                elif kind == "p2mm":
                    b = op[1]
                    eng.matmul(p2s2b[:, :64],
                               lhsT=h1T_sb[:, b * 128:(b + 1) * 128],
                               rhs=w2p_sb[:], start=True, stop=True
                               ).then_inc(p_s, 1)
                    eng.matmul(p2s2b[:, 64:128],
                               lhsT=h1T_sb[:, b * 128:(b + 1) * 128],
                               rhs=w2s_sb[:], start=True, stop=True
                               ).then_inc(p_s, 1)
                elif kind == "p2cp":
                    b = op[1]
                    eng.activation(p2nm_sb[:, b, :], p2s2b[:, :64],
                                   AF.Copy).then_inc(a_s, 1)
                elif kind == "s2cp":
                    b = op[1]
                    eng.activation(s2_sb[:, b, :], p2s2b[:, 64:128],
                                   AF.Copy).then_inc(a_s, 1)
                elif kind == "p2wr":
                    b = op[1]
                    eng.dma_start(out=p2_loc[b * 128:(b + 1) * 128, :],
                                  in_=p2nm_sb[:, b, :]).then_inc(
                        wr_sems[piece_of_block[b]], 16)
                elif kind == "s2bias":
                    eng.tensor_tensor(
                        out=s2_sb[:], in0=s2_sb[:],
                        in1=b2_sb[:, None, :].to_broadcast([128, NB, 64]),
                        op=ALU.add).then_inc(v_s, 1)
                elif kind == "ag":
                    _, which, k = op
                    loc = p2_loc if which == "p2" else h2_loc
                    tab = p2t if which == "p2" else h2t
                    eng.collective_compute(
                        "AllGather", ALU.bypass,
                        replica_groups=[list(range(NCORES))],
                        ins=[loc[locbase[k]:locbase[k] + pn[k] * 128, :]],
                        outs=[tab[pairbase[k]:pairbase[k + 1], :]],
                    ).then_inc(cc_s, 1)
                elif kind == "h2add":
                    b = op[1]
                    eng.tensor_tensor(out=h2pre_sb[:, b % 2, :],
                                      in0=aggreg(b, 64),
                                      in1=s2_sb[:, b, :],
                                      op=ALU.add).then_inc(v_s, 1)
                elif kind == "h2relu":
                    b = op[1]
                    eng.activation(h2nm_sb[:, b, :], h2pre_sb[:, b % 2, :],
                                   AF.Relu).then_inc(a_s, 1)
                elif kind == "h2wr":
                    b = op[1]
                    eng.dma_start(out=h2_loc[b * 128:(b + 1) * 128, :],
                                  in_=h2nm_sb[:, b, :]).then_inc(
                        wr_sems[piece_of_block[b]], 16)
                elif kind == "p3tr":
                    _, g, wi, tb = op
                    eng.transpose(
                        out=aggps[tb][:].bitcast(BF16)[:, wi * 128:(wi + 1) * 128],
                        in_=oh_sb[:, g % OHS, :],
                        identity=ident_sb[:]).then_inc(p_s, 1)
                elif kind == "p3ocp":
                    _, w, nb = op
                    eng.activation(
                        osb_sb[:, w % 2, :nb * 128],
                        aggps[w % 2][:].bitcast(BF16)[:, :nb * 128],
                        AF.Copy).then_inc(a_s, 1)
                elif kind == "p3exp":
                    _, g, wi, eb, b = op
                    ebt = h1b if eb == 0 else p2s2b
                    eng.matmul(ebt[:, wi * 64:(wi + 1) * 64],
                               lhsT=osb_sb[:, eb, wi * 128:(wi + 1) * 128],
                               rhs=h2nm_sb[:, b, :], start=True, stop=True
                               ).then_inc(p_s, 1)
                elif kind == "p3mult":
                    _, w, gw0, nb, slot, g0s, hi = op
                    pr = slice(64, 128) if hi else slice(0, 64)
                    cr = slice(64, 128) if hi else slice(0, 64)
                    j0 = gw0 - g0s
                    ebt = h1b if w % 2 == 0 else p2s2b
                    eng.tensor_tensor(
                        out=prod_sb[pr, w % 2, :nb, :],
                        in0=ebt[pr, :nb * 64].rearrange(
                            "p (g f) -> p g f", f=64),
                        in1=gbuf[pr, slot, j0 * 128:(j0 + nb) * 128].rearrange(
                            "p (g f) -> p g f", f=128)[:, :, cr],
                        op=ALU.mult).then_inc(v_s, 1)
                elif kind == "p3red":
                    _, w, gw0, nb = op
                    eng.reduce_sum(out=dots_sb[:, gw0:gw0 + nb],
                                   in_=prod_sb[:, w % 2, :nb, :],
                                   axis=mybir.AxisListType.X).then_inc(v_s, 1)
                elif kind == "sigmoid":
                    eng.activation(dots_sb[:], dots_sb[:], AF.Sigmoid
                                   ).then_inc(a_s, 1)
                elif kind == "sxwr":
                    eng.dma_start(out=sx_out[:], in_=dots_sb[:]
                                  ).then_inc(dma_s, 16)
                else:
                    raise ValueError(kind)

        @block.sync
        def _(e):
            run_ops(e, "sp")

        @block.gpsimd
        def _(e):
            run_ops(e, "pool")

        @block.vector
        def _(e):
            run_ops(e, "dve")

        @block.scalar
        def _(e):
            run_ops(e, "act")

        @block.tensor
        def _(e):
            run_ops(e, "pe")

    nc.compile()
    return nc


# ---------------------------------------------------------------- host glue
def host_prep(X, edge_row, edge_col, edge_vals, W1p, b1p, W1s, b1s,
              W2p, b2p, W2s, b2s, plan):
    p = plan
    NP, NPc = p.NP, p.NPc
    Xp = np.zeros((NP, X.shape[1]), np.float32)
    Xp[: X.shape[0]] = X
    Xperm = Xp[p.perm]                       # (c,b,l)-ordered
    Xtab = Xp[p.node_of_trow]                # table-row ordered
    Xg = Xtab.astype(ml_dtypes.bfloat16)
    b1 = np.ascontiguousarray((b1p + b1s).astype(np.float32)[:, None])
    b2rep = np.ascontiguousarray(
        np.tile((b2p + b2s).astype(np.float32)[None, :], (128, 1)))
    in_maps = []
    for c in range(NCORES):
        in_maps.append({
            "xg": Xg,
            "xlT": np.ascontiguousarray(
                Xperm[c * NPc:(c + 1) * NPc].T).astype(ml_dtypes.bfloat16),
            "idx1": wrap_idx(p.idx1[c]),
            "idx2": wrap_idx(p.idx2[c]),
            "dloc": colmajor(p.dloc[c]),
            "val": colmajor(p.val[c]),
            "vale": colmajor(p.val[c]) * p.parmask[:, None],
            "valo": colmajor(p.val[c]) * (1.0 - p.parmask)[:, None],
            "w1p": np.ascontiguousarray(W1p).astype(ml_dtypes.bfloat16),
            "w1s": np.ascontiguousarray(W1s).astype(ml_dtypes.bfloat16),
            "w2p": np.ascontiguousarray(W2p).astype(ml_dtypes.bfloat16),
            "w2s": np.ascontiguousarray(W2s).astype(ml_dtypes.bfloat16),
            "b1": b1, "b2rep": b2rep,
        })
    return in_maps


def unpermute_sx(results, plan, n_edges):
    p = plan
    sx = np.empty(n_edges, np.float32)
    for c in range(NCORES):
        flat = results[c]["sx"].T.reshape(-1)
        m = p.core_of_edge[:n_edges] == c
        sx[m] = flat[p.slot_of_edge[m]]
    return sx


_CACHE = {}


def kernel(X, edge_row, edge_col, edge_vals,
           W_pass1, b_pass1, W_self1, b_self1,
           W_pass2, b_pass2, W_self2, b_self2):
    X = np.asarray(X, np.float32)
    er = np.asarray(edge_row).astype(np.int64)
    ec = np.asarray(edge_col).astype(np.int64)
    ev_ = np.asarray(edge_vals, np.float32)
    n_nodes, n_edges = X.shape[0], len(er)

    key = (n_nodes, n_edges, int(er[0]), int(ec[0]))
    if key not in _CACHE:
        plan = plan_graph(er, ec, ev_, n_nodes)
        nc = build(plan)
        _CACHE[key] = (plan, nc)
    plan, nc = _CACHE[key]

    in_maps = host_prep(X, er, ec, ev_,
                        np.asarray(W_pass1), np.asarray(b_pass1),
                        np.asarray(W_self1), np.asarray(b_self1),
                        np.asarray(W_pass2), np.asarray(b_pass2),
                        np.asarray(W_self2), np.asarray(b_self2), plan)
    res = run_bass_kernel_spmd(nc, in_maps, core_ids=list(range(NCORES)))
    return unpermute_sx(res.results, plan, n_edges)


# revision 16
# speedup vs baseline: 1.0251x; 1.0098x over previous
"""Self-contained GCN edge-dot kernel for 8 TRN2 NeuronCores (v2).

kernel(**inputs) takes the FULL problem inputs and returns sigmoid edge
scores for every edge, computed SPMD across 8 cores with bass/bacc.

Strategy vs v1: nodes assigned degree-balanced to (core, block) with a
piece-major gather-table numbering so the two inter-layer exchanges
(P2 = H1 @ W_pass2 and H2, both 64-wide bf16 packed two-nodes-per-256B
row) stream as per-piece AllGather collectives overlapped with compute;
one-hot selection matrices built per group on DVE/Pool via
tensor_scalar (is_equal x val, 4x DVE mode); phase-3 edge dot uses
PE transpose + batched ACT psum copy + batched DVE mult/reduce; edge
val folded into the one-hot so gathered rows are used unscaled.
"""
import sys
sys.path.insert(0, "/opt/trn_rl_repo")
import numpy as np
import ml_dtypes
import concourse.bass as bass
import concourse.bacc as bacc
import concourse.mybir as mybir
from concourse import masks
from concourse.bass_utils import run_bass_kernel_spmd

F32 = mybir.dt.float32
BF16 = mybir.dt.bfloat16
I16 = mybir.dt.int16
AF = mybir.ActivationFunctionType
ALU = mybir.AluOpType
NCORES = 8


# ---------------------------------------------------------------- host planning
class Plan:
    pass


def plan_graph(edge_row, edge_col, edge_vals, n_nodes, NB=49, CB=4,
               pstart=(0, 25)):
    p = Plan()
    NPc = NB * 128
    NP = NPc * NCORES
    assert n_nodes <= NP
    NPIECES = len(pstart)
    pstart = list(pstart) + [NB]
    pn = [pstart[k + 1] - pstart[k] for k in range(NPIECES)]
    p.NB, p.NPc, p.NP, p.CB, p.NPIECES = NB, NPc, NP, CB, NPIECES
    p.pn = pn

    E = len(edge_row)
    deg = np.bincount(edge_row, minlength=NP)
    order = np.argsort(-deg, kind="stable")
    nblocks = NCORES * NB
    newpos = np.empty(NP, np.int64)
    for g in range(nblocks):
        members = order[g::nblocks]
        c, b = g // NB, g % NB
        newpos[members] = c * NPc + b * 128 + np.arange(len(members))
    p.newpos = newpos
    perm = np.empty(NP, np.int64)
    perm[newpos] = np.arange(NP)
    p.perm = perm

    # piece-major gather-table row numbering: trow(c,b,l) for piece k
    #   = trowbase[k] + c*pn[k]*128 + (b - pstart[k])*128 + l
    trowbase = np.zeros(NPIECES + 1, np.int64)
    for k in range(NPIECES):
        trowbase[k + 1] = trowbase[k] + NCORES * pn[k] * 128
    p.trowbase = trowbase
    pos_c = newpos // NPc
    pos_b = (newpos % NPc) // 128
    pos_l = newpos % 128
    pn_arr = np.array(pn)
    pstart_arr = np.array(pstart[:NPIECES])
    pk = np.searchsorted(pstart_arr[1:] if NPIECES > 1 else np.array([NB]),
                         pos_b, side="right")
    pk = np.minimum(pk, NPIECES - 1)
    trow_of_node = (trowbase[pk] + pos_c * pn_arr[pk] * 128 +
                    (pos_b - pstart_arr[pk]) * 128 + pos_l)
    p.trow_of_node = trow_of_node  # original node id -> table row
    # inverse: table row -> original node id
    node_of_trow = np.empty(NP, np.int64)
    node_of_trow[trow_of_node] = np.arange(NP)
    p.node_of_trow = node_of_trow

    nr = newpos[edge_row]          # dest in (c,b,l) space
    core = nr // NPc
    blk = (nr % NPc) // 128
    dloc = nr % 128
    strow = trow_of_node[edge_col]  # source table row
    spiece = np.searchsorted(trowbase[1:NPIECES + 1], strow, side="right")
    sidx1 = strow - trowbase[spiece]            # idx within piece (p1, rows)
    sidx2 = sidx1 // 2                          # idx within piece (p2/p3 pairs)
    par = (strow % 2).astype(np.int64)          # parity within pair row

    # bucket (core, block, piece) -> even/odd edge lists
    G = np.zeros((NB, NPIECES), np.int64)
    buckets = {}
    for c in range(NCORES):
        m_c = core == c
        for b in range(NB):
            m_b = m_c & (blk == b)
            for k in range(NPIECES):
                m = m_b & (spiece == k)
                ev_ = np.nonzero(m & (par == 0))[0]
                od_ = np.nonzero(m & (par == 1))[0]
                buckets[(c, b, k)] = (ev_, od_)
                ng = max(-(-len(ev_) // 64), -(-len(od_) // 64))
                G[b, k] = max(G[b, k], ng)
    for b in range(NB):
        if G[b].sum() == 0:
            G[b, 0] = 1
    p.G = G
    p.Gtot = int(G.sum())
    S = p.Gtot * 128

    chunks = [list(range(i, min(i + CB, NB))) for i in range(0, NB, CB)]
    p.chunks = chunks
    segs = []
    gidx = 0
    firstg = {}
    lastg = {}
    for ci, cblocks in enumerate(chunks):
        for k in range(NPIECES):
            for b in cblocks:
                ng = int(G[b, k])
                if ng == 0:
                    continue
                segs.append((ci, k, b, gidx, ng))
                if b not in firstg:
                    firstg[b] = gidx
                lastg[b] = gidx + ng - 1
                gidx += ng
    assert gidx == p.Gtot
    p.segs = segs
    p.firstg, p.lastg = firstg, lastg
    # gather segments: one dma_gather per (ci, k) covering its blocks' groups
    gsegs = []
    i = 0
    while i < len(segs):
        ci, k, b, g0, ng = segs[i]
        j = i
        tot = 0
        while j < len(segs) and segs[j][0] == ci and segs[j][1] == k:
            tot += segs[j][4]
            j += 1
        gsegs.append((ci, k, g0, tot))
        i = j
    p.gsegs = gsegs
    p.GH = max(t for (_, _, _, t) in gsegs)
    p.block_of_g = {}
    for (ci, k, b, g0, ng) in segs:
        for g in range(g0, g0 + ng):
            p.block_of_g[g] = b

    p.idx1 = np.zeros((NCORES, S), np.int16)
    p.idx2 = np.zeros((NCORES, S), np.int16)
    p.dloc = np.zeros((NCORES, S), np.float32)
    p.val = np.zeros((NCORES, S), np.float32)
    p.parmask = (np.arange(128) < 64).astype(np.float32)
    p.slot_of_edge = np.full(E, -1, np.int64)
    p.core_of_edge = core
    for c in range(NCORES):
        for (ci, k, b, g0, ng) in segs:
            ev_, od_ = buckets[(c, b, k)]
            for half, eids in ((0, ev_), (1, od_)):
                n = len(eids)
                assert n <= ng * 64
                gi = np.arange(n) // 64
                sl = (g0 + gi) * 128 + half * 64 + np.arange(n) % 64
                p.idx1[c, sl] = sidx1[eids]
                p.idx2[c, sl] = sidx2[eids]
                p.dloc[c, sl] = dloc[eids]
                p.val[c, sl] = edge_vals[eids]
                p.slot_of_edge[eids] = sl
    return p


def wrap_idx(idx_flat):
    S = len(idx_flat)
    w = idx_flat.reshape(S // 16, 16).T
    return np.tile(w, (8, 1)).copy()


def colmajor(a):
    S = len(a)
    return a.reshape(S // 128, 128).T.copy()


# ---------------------------------------------------------------- emission fw
class Counters:
    def __init__(self):
        self.val = {}
        self.last = {}

    def inc(self, sem, by):
        self.val[sem] = self.val.get(sem, 0) + by
        return self.val[sem]

    def cur(self, sem):
        return self.val.get(sem, 0)

    def wait(self, eng_ops, eng_name, sem, v):
        if v <= 0:
            return
        key = (eng_name, sem)
        if self.last.get(key, -1) >= v:
            return
        self.last[key] = v
        eng_ops.append(("wait", sem, v))


DMA, V, A, P, PL, CC = "dma", "v", "a", "p", "pl", "cc"
GTS = ("gt0", "gt1", "gt2", "gt3")
WRS = ("wr0", "wr1")


def build(plan):
    p = plan
    NB, NPc, NP, CB = p.NB, p.NPc, p.NP, p.CB
    NPIECES, pn, G = p.NPIECES, p.pn, p.G
    Gtot, segs, gsegs, chunks = p.Gtot, p.segs, p.gsegs, p.chunks
    firstg, lastg, block_of_g = p.firstg, p.lastg, p.block_of_g
    S = Gtot * 128
    GH = p.GH
    NSLOT = 4      # gather buffer slots
    OHS = 32       # one-hot slots

    nc = bacc.Bacc()
    dp = nc.declare_dram_parameter
    xg = dp("xg", [NP, 128], BF16, isOutput=False)
    xlT_in = dp("xlT", [128, NPc], BF16, isOutput=False)
    idx1_in = dp("idx1", [128, S // 16], I16, isOutput=False)
    idx2_in = dp("idx2", [128, S // 16], I16, isOutput=False)
    dloc_in = dp("dloc", [128, Gtot], F32, isOutput=False)
    val_in = dp("val", [128, Gtot], F32, isOutput=False)
    vale_in = dp("vale", [128, Gtot], F32, isOutput=False)
    valo_in = dp("valo", [128, Gtot], F32, isOutput=False)
    w1p_in = dp("w1p", [128, 128], BF16, isOutput=False)
    w1s_in = dp("w1s", [128, 128], BF16, isOutput=False)
    w2p_in = dp("w2p", [128, 64], BF16, isOutput=False)
    w2s_in = dp("w2s", [128, 64], BF16, isOutput=False)
    b1_in = dp("b1", [128, 1], F32, isOutput=False)
    b2_in = dp("b2rep", [128, 64], F32, isOutput=False)
    sx_out = dp("sx", [128, Gtot], F32, isOutput=True)

    p2_loc = nc.dram_tensor("p2_loc", [NPc, 64], BF16)
    h2_loc = nc.dram_tensor("h2_loc", [NPc, 64], BF16)
    p2t = nc.dram_tensor("p2t", [NP // 2, 128], BF16, addr_space="Shared")
    h2t = nc.dram_tensor("h2t", [NP // 2, 128], BF16, addr_space="Shared")

    # piece boundaries in table spaces
    trowbase = p.trowbase
    pairbase = [int(trowbase[k] // 2) for k in range(NPIECES + 1)]
    locbase = []   # local p2_loc/h2_loc row range per piece
    acc = 0
    for k in range(NPIECES):
        locbase.append(acc)
        acc += pn[k] * 128

    ops = {e: [] for e in ("sp", "pool", "dve", "act", "pe")}
    C = Counters()
    ev = {}
    sp, pool, dve, act, pe = (ops[k] for k in ("sp", "pool", "dve", "act", "pe"))

    # ---- loads
    _ld_names = ("idx1", "idx2", "dloc", "val", "vale", "valo", "xlT",
                 "w1p", "w1s", "w2p", "w2s", "b1", "b2")
    for name in _ld_names:
        sp.append(("dma_sb", name))
        C.inc(DMA, 16)
    for name in _ld_names:
        ev["ld_" + name] = (DMA, C.cur(DMA))
    pool.append(("iota",))
    pool.append(("ident",))
    ev["p0_pool"] = (PL, C.inc(PL, 1))

    # last p1/p2 block using psum bank j (for bank-reuse waits)
    last_user = {}
    for b in range(NB):
        last_user[b % CB] = b
    pstart_list = [0]
    for k in range(NPIECES - 1):
        pstart_list.append(pstart_list[-1] + pn[k])
    piece_of_block = {}
    for b in range(NB):
        piece_of_block[b] = max(k for k in range(NPIECES)
                                if pstart_list[k] <= b)

    # ---------------- gather emission helper
    gseq = [0]

    def emit_gather(ph, ci, k, g0, gsum):
        s = gseq[0]
        slot = s % NSLOT
        # slot reuse: wait consumers of gather s-NSLOT
        prev = s - NSLOT
        if prev >= 0:
            pph, pg0, pgsum = gmeta[prev]
            if pph == "p3":
                C.wait(pool, "pool", V, ev[f"p3_multdone_g{pg0 + pgsum - 1}"][1])
            else:
                C.wait(pool, "pool", P, ev[f"{pph}_agg_g{pg0 + pgsum - 1}"][1])
        C.wait(pool, "pool", DMA, ev["ld_idx1" if ph == "p1" else "ld_idx2"][1])
        if ph == "p2":
            C.wait(pool, "pool", CC, k + 1)
        elif ph == "p3":
            C.wait(pool, "pool", CC, NPIECES + k + 1)
        pool.append(("gather", ph, k, g0, gsum, slot))
        gs = GTS[slot]
        ev[f"{ph}_gather_g{g0}"] = (gs, C.inc(gs, 16))
        gmeta.append((ph, g0, gsum))
        gseq[0] += 1
        return slot

    gmeta = []

    # ---------------- p1 / p2 aggregation phases
    def emit_agg_phase(ph):
        for (ci, k, g0s, gsum) in gsegs:
            slot = emit_gather(ph, ci, k, g0s, gsum)
            segs_here = [(b, g0, ng) for (ci2, k2, b, g0, ng) in segs
                         if ci2 == ci and k2 == k]
            for (b, g0, ng) in segs_here:
                for g in range(g0, g0 + ng):
                    # weighted one-hot build(s)
                    if ph == "p1":
                        C.wait(dve, "dve", DMA, ev["ld_val"][1])
                        C.wait(dve, "dve", PL, ev["p0_pool"][1])
                        if g - OHS >= 0:
                            C.wait(dve, "dve", P, ev[f"p1_agg_g{g - OHS}"][1])
                        dve.append(("ohw", g, "val", 0))  # slot g%OHS
                        ev[f"p1_oh_g{g}"] = (V, C.inc(V, 1))
                    else:
                        # two parity-masked builds: even on DVE, odd on POOL
                        # p2 uses oh slots (2g)%OHS and (2g+1)%OHS
                        lue = (2 * g) % OHS + OHS * ((Gtot - 1 - (2 * g) % OHS) // OHS)
                        luo = (2 * g + 1) % OHS + OHS * ((Gtot - 1 - (2 * g + 1) % OHS) // OHS)
                        C.wait(dve, "dve", DMA, ev["ld_vale"][1])
                        C.wait(dve, "dve", P, ev[f"p1_agg_g{lue}"][1])
                        if g - OHS // 2 >= 0:
                            C.wait(dve, "dve", P, ev[f"p2_agg_g{g - OHS // 2}"][1])
                        dve.append(("ohw", g, "vale", 0))  # slot (2g)%OHS
                        ev[f"p2_ohe_g{g}"] = (V, C.inc(V, 1))
                        C.wait(pool, "pool", DMA, ev["ld_valo"][1])
                        C.wait(pool, "pool", P, ev[f"p1_agg_g{luo}"][1])
                        if g - OHS // 2 >= 0:
                            C.wait(pool, "pool", P, ev[f"p2_agg_g{g - OHS // 2}"][1])
                        pool.append(("ohw", g, "valo", 1))
                        ev[f"p2_oho_g{g}"] = (PL, C.inc(PL, 1))

                    # PE: aggregation matmul(s)
                    _gs, _gv = ev[f"{ph}_gather_g{g0s}"]
                    C.wait(pe, "pe", _gs, _gv)
                    if ph == "p1":
                        C.wait(pe, "pe", V, ev[f"p1_oh_g{g}"][1])
                    else:
                        C.wait(pe, "pe", V, ev[f"p2_ohe_g{g}"][1])
                        C.wait(pe, "pe", PL, ev[f"p2_oho_g{g}"][1])
                    first = g == firstg[b]
                    last = g == lastg[b]
                    if first:
                        # psum bank reuse
                        if ph == "p1":
                            if b - CB >= 0:
                                C.wait(pe, "pe", A, ev[f"p1_aggcopy_b{b - CB}"][1])
                        else:
                            lb = last_user[b % CB]
                            C.wait(pe, "pe", A, ev[f"p1_aggcopy_b{lb}"][1])
                            if b - CB >= 0:
                                C.wait(pe, "pe", V, ev[f"p2_h2add_b{b - CB}"][1])
                    j = g - g0s
                    if ph == "p1":
                        pe.append(("agg1", b, g, j, slot, first, last))
                        ev[f"p1_agg_g{g}"] = (P, C.inc(P, 1))
                    else:
                        pe.append(("agg2", b, g, j, slot, first, last))
                        ev[f"p2_agg_g{g}"] = (P, C.inc(P, 2))
                    if last:
                        ev[f"{ph}_aggstop_b{b}"] = (P, C.cur(P))
                        if ph == "p1":
                            p1_block_tail(b)
                        else:
                            p2_block_tail(b)

    def p1_block_tail(b):
        # ACT: aggcopy psum->sbuf bf16
        C.wait(act, "act", P, ev[f"p1_aggstop_b{b}"][1])
        if b - 2 >= 0:
            C.wait(act, "act", P, ev[f"p1_h1mm_b{b - 2}"][1])
        act.append(("aggcopy", b))
        ev[f"p1_aggcopy_b{b}"] = (A, C.inc(A, 1))
        # PE: h1 = relu(W1p^T aggT + W1s^T xlT + b1)
        C.wait(pe, "pe", DMA, ev["ld_xlT"][1])
        C.wait(pe, "pe", A, ev[f"p1_aggcopy_b{b}"][1])
        if b - 1 >= 0:
            C.wait(pe, "pe", A, ev[f"p1_h1relu_b{b - 1}"][1])
        pe.append(("h1mm", b))
        ev[f"p1_h1mm_b{b}"] = (P, C.inc(P, 2))
        C.wait(act, "act", P, ev[f"p1_h1mm_b{b}"][1])
        C.wait(act, "act", DMA, ev["ld_b1"][1])
        act.append(("h1relu", b))
        ev[f"p1_h1relu_b{b}"] = (A, C.inc(A, 1))
        # PE: P2 block + S2 block
        C.wait(pe, "pe", DMA, ev["ld_w2s"][1])
        C.wait(pe, "pe", A, ev[f"p1_h1relu_b{b}"][1])
        if b - 1 >= 0:
            C.wait(pe, "pe", A, ev[f"p1_s2cp_b{b - 1}"][1])
        pe.append(("p2mm", b))
        ev[f"p1_p2mm_b{b}"] = (P, C.inc(P, 2))
        C.wait(act, "act", P, ev[f"p1_p2mm_b{b}"][1])
        act.append(("p2cp", b))
        ev[f"p1_p2cp_b{b}"] = (A, C.inc(A, 1))
        act.append(("s2cp", b))
        ev[f"p1_s2cp_b{b}"] = (A, C.inc(A, 1))
        C.wait(sp, "sp", A, ev[f"p1_p2cp_b{b}"][1])
        wk = WRS[piece_of_block[b]]
        C.wait(sp, "sp", wk, C.cur(wk))
        sp.append(("p2wr", b))
        ev[f"p1_p2wr_b{b}"] = (wk, C.inc(wk, 16))

    def p2_block_tail(b):
        C.wait(dve, "dve", P, ev[f"p2_aggstop_b{b}"][1])
        C.wait(dve, "dve", V, ev["s2bias"][1])
        if b - 2 >= 0:
            C.wait(dve, "dve", A, ev[f"p2_h2relu_b{b - 2}"][1])
        dve.append(("h2add", b))
        ev[f"p2_h2add_b{b}"] = (V, C.inc(V, 1))
        C.wait(act, "act", V, ev[f"p2_h2add_b{b}"][1])
        act.append(("h2relu", b))
        ev[f"p2_h2relu_b{b}"] = (A, C.inc(A, 1))
        C.wait(sp, "sp", A, ev[f"p2_h2relu_b{b}"][1])
        wk = WRS[piece_of_block[b]]
        C.wait(sp, "sp", wk, C.cur(wk))
        sp.append(("h2wr", b))
        ev[f"p2_h2wr_b{b}"] = (wk, C.inc(wk, 16))

    # ================= PHASE 1 =================
    emit_agg_phase("p1")

    # s2bias after all s2cp
    C.wait(dve, "dve", A, ev[f"p1_s2cp_b{NB - 1}"][1])
    C.wait(dve, "dve", DMA, ev["ld_b2"][1])
    dve.append(("s2bias",))
    ev["s2bias"] = (V, C.inc(V, 1))

    # AG_p2 pieces (pool stream, after p1 gathers)
    for k in range(NPIECES):
        lastb = sum(pn[:k + 1]) - 1
        wk, wv = ev[f"p1_p2wr_b{lastb}"]
        C.wait(pool, "pool", wk, wv)
        pool.append(("ag", "p2", k))
        ev[f"ag_p2_{k}"] = (CC, C.inc(CC, 1))

    # ================= PHASE 2 =================
    emit_agg_phase("p2")

    # AG_h2 pieces
    for k in range(NPIECES):
        lastb = sum(pn[:k + 1]) - 1
        wk, wv = ev[f"p2_h2wr_b{lastb}"]
        C.wait(pool, "pool", wk, wv)
        pool.append(("ag", "h2", k))
        ev[f"ag_h2_{k}"] = (CC, C.inc(CC, 1))

    # ================= PHASE 3 =================
    # windows of <=8 groups within each gather segment
    wseq = [0]
    pending_expand = []

    def p3_emit_window(w, gw0, nb, slot, g0s):
        # pool: one-hot builds (unweighted)
        for g in range(gw0, gw0 + nb):
            C.wait(pool, "pool", DMA, ev["ld_dloc"][1])
            C.wait(pool, "pool", P, ev[f"p2_agg_g{Gtot - 1}"][1])
            if g - OHS >= 0:
                C.wait(pool, "pool", P, ev[f"p3_tr_g{g - OHS}"][1])
            pool.append(("ohu", g))
            ev[f"p3_oh_g{g}"] = (PL, C.inc(PL, 1))
        # PE: transposes into tb bank (w%2)
        for wi, g in enumerate(range(gw0, gw0 + nb)):
            C.wait(pe, "pe", PL, ev[f"p3_oh_g{g}"][1])
            if w - 2 >= 0:
                C.wait(pe, "pe", A, ev[f"p3_ocp_w{w - 2}"][1])
            if w < 2:
                C.wait(pe, "pe", V, ev[f"p2_h2add_b{NB - 1}"][1])
            pe.append(("p3tr", g, wi, w % 2))
            ev[f"p3_tr_g{g}"] = (P, C.inc(P, 1))
        ev[f"p3_trdone_w{w}"] = (P, C.cur(P))
        # ACT: batched copy
        C.wait(act, "act", P, ev[f"p3_trdone_w{w}"][1])
        if w - 2 >= 0:
            C.wait(act, "act", P, ev[f"p3_expdone_w{w - 2}"][1])
        act.append(("p3ocp", w, nb))
        ev[f"p3_ocp_w{w}"] = (A, C.inc(A, 1))
        # PE: expands (deferred one window for pipelining)
        pending_expand.append((w, gw0, nb, slot, g0s))
        if len(pending_expand) > 1:
            p3_emit_expand(*pending_expand.pop(0))

    def p3_emit_expand(w, gw0, nb, slot, g0s):
        for wi, g in enumerate(range(gw0, gw0 + nb)):
            C.wait(pe, "pe", A, ev[f"p3_ocp_w{w}"][1])
            if w - 2 >= 0:
                C.wait(pe, "pe", V, ev[f"p3_multdone_w{w - 2}"][1])
            if w < 2:
                C.wait(pe, "pe", A, ev[f"p1_h1relu_b{NB - 1}"][1])
                C.wait(pe, "pe", A, ev[f"p1_s2cp_b{NB - 1}"][1])
            pe.append(("p3exp", g, wi, w % 2, block_of_g[g]))
            ev[f"p3_exp_g{g}"] = (P, C.inc(P, 1))
        ev[f"p3_expdone_w{w}"] = (P, C.cur(P))
        # DVE: batched mult lo/hi + reduce
        C.wait(dve, "dve", P, ev[f"p3_expdone_w{w}"][1])
        _gs, _gv = ev[f"p3_gather_g{g0s}"]
        C.wait(dve, "dve", _gs, _gv)
        if w - 2 >= 0:
            C.wait(dve, "dve", V, ev[f"p3_reddone_w{w - 2}"][1])
        dve.append(("p3mult", w, gw0, nb, slot, g0s, 0))
        dve.append(("p3mult", w, gw0, nb, slot, g0s, 1))
        mv = C.inc(V, 2)
        ev[f"p3_multdone_w{w}"] = (V, mv)
        for g in range(gw0, gw0 + nb):
            ev[f"p3_multdone_g{g}"] = (V, mv)
        C.wait(dve, "dve", V, ev[f"p3_multdone_w{w}"][1])
        dve.append(("p3red", w, gw0, nb))
        ev[f"p3_reddone_w{w}"] = (V, C.inc(V, 1))

    for (ci, k, g0s, gsum) in gsegs:
        slot = emit_gather("p3", ci, k, g0s, gsum)
        g = g0s
        while g < g0s + gsum:
            nb = min(16, g0s + gsum - g)
            p3_emit_window(wseq[0], g, nb, slot, g0s)
            wseq[0] += 1
            g += nb
    while pending_expand:
        p3_emit_expand(*pending_expand.pop(0))

    # sigmoid + writeback
    lastw = wseq[0] - 1
    C.wait(act, "act", V, ev[f"p3_reddone_w{lastw}"][1])
    act.append(("sigmoid",))
    ev["sig"] = (A, C.inc(A, 1))
    C.wait(sp, "sp", A, ev["sig"][1])
    sp.append(("sxwr",))
    C.inc(DMA, 16)

    # ------------------------------------------------ emit to bass
    from contextlib import ExitStack
    _es = ExitStack()
    with _es:
        idx1_sb = _es.enter_context(nc.sbuf_tensor("idx1_sb", [128, S // 16], I16))
        idx2_sb = _es.enter_context(nc.sbuf_tensor("idx2_sb", [128, S // 16], I16))
        dloc_sb = _es.enter_context(nc.sbuf_tensor("dloc_sb", [128, Gtot], F32))
        val_sb = _es.enter_context(nc.sbuf_tensor("val_sb", [128, Gtot], F32))
        vale_sb = _es.enter_context(nc.sbuf_tensor("vale_sb", [128, Gtot], F32))
        valo_sb = _es.enter_context(nc.sbuf_tensor("valo_sb", [128, Gtot], F32))
        xlT_sb = _es.enter_context(nc.sbuf_tensor("xlT_sb", [128, NPc], BF16))
        w1p_sb = _es.enter_context(nc.sbuf_tensor("w1p_sb", [128, 128], BF16))
        w1s_sb = _es.enter_context(nc.sbuf_tensor("w1s_sb", [128, 128], BF16))
        w2p_sb = _es.enter_context(nc.sbuf_tensor("w2p_sb", [128, 64], BF16))
        w2s_sb = _es.enter_context(nc.sbuf_tensor("w2s_sb", [128, 64], BF16))
        b1_sb = _es.enter_context(nc.sbuf_tensor("b1_sb", [128, 1], F32))
        b2_sb = _es.enter_context(nc.sbuf_tensor("b2_sb", [128, 64], F32))
        iota_sb = _es.enter_context(nc.sbuf_tensor("iota_sb", [128, 128], BF16))
        ident_sb = _es.enter_context(nc.sbuf_tensor("ident_sb", [128, 128], BF16))
        oh_sb = _es.enter_context(nc.sbuf_tensor("oh_sb", [128, OHS, 128], BF16))
        gbuf = _es.enter_context(
            nc.sbuf_tensor("gbuf", [128, NSLOT, GH * 128], BF16))
        h1T_sb = _es.enter_context(nc.sbuf_tensor("h1T_sb", [128, NPc], BF16))
        aggT_sb = _es.enter_context(nc.sbuf_tensor("aggT_sb", [128, 2, 128], BF16))
        s2_sb = _es.enter_context(nc.sbuf_tensor("s2_sb", [128, NB, 64], F32))
        h2nm_sb = _es.enter_context(nc.sbuf_tensor("h2nm_sb", [128, NB, 64], BF16))
        p2nm_sb = _es.enter_context(nc.sbuf_tensor("p2nm_sb", [128, NB, 64], BF16))
        h2pre_sb = _es.enter_context(nc.sbuf_tensor("h2pre_sb", [128, 2, 64], F32))
        osb_sb = _es.enter_context(nc.sbuf_tensor("osb_sb", [128, 2, 16 * 128], BF16))
        prod_sb = _es.enter_context(nc.sbuf_tensor("prod_sb", [128, 2, 16, 64], F32))
        dots_sb = _es.enter_context(nc.sbuf_tensor("dots_sb", [128, Gtot], F32))
        aggps = [_es.enter_context(nc.psum_tensor(f"aggps{j}", [128, 1024], F32))
                 for j in range(2)]
        h1b = _es.enter_context(nc.psum_tensor("h1b", [128, 1024], F32))
        p2s2b = _es.enter_context(nc.psum_tensor("p2s2b", [128, 1024], F32))

        def aggreg(b, w):
            t = aggps[(b % 4) // 2]
            c0 = ((b % 4) % 2) * 512
            return t[:, c0:c0 + w]
        dma_s = _es.enter_context(nc.semaphore("dma_s"))
        gt_sems = [_es.enter_context(nc.semaphore(f"gt{j}_s"))
                   for j in range(NSLOT)]
        wr_sems = [_es.enter_context(nc.semaphore(f"wr{j}_s"))
                   for j in range(NPIECES)]
        v_s = _es.enter_context(nc.semaphore("v_s"))
        a_s = _es.enter_context(nc.semaphore("a_s"))
        p_s = _es.enter_context(nc.semaphore("p_s"))
        pl_s = _es.enter_context(nc.semaphore("pl_s"))
        cc_s = _es.enter_context(nc.semaphore("cc_s"))
        block = _es.enter_context(nc.Block())
        sems = {DMA: dma_s, V: v_s, A: a_s, P: p_s, PL: pl_s, CC: cc_s}
        for j in range(NSLOT):
            sems[GTS[j]] = gt_sems[j]
        for j in range(NPIECES):
            sems[WRS[j]] = wr_sems[j]

        sb_map = {"idx1": idx1_sb, "idx2": idx2_sb, "dloc": dloc_sb,
                  "val": val_sb, "vale": vale_sb, "valo": valo_sb,
                  "xlT": xlT_sb, "w1p": w1p_sb, "w1s": w1s_sb,
                  "w2p": w2p_sb, "w2s": w2s_sb, "b1": b1_sb, "b2": b2_sb}
        in_map = {"idx1": idx1_in, "idx2": idx2_in, "dloc": dloc_in,
                  "val": val_in, "vale": vale_in, "valo": valo_in,
                  "xlT": xlT_in, "w1p": w1p_in, "w1s": w1s_in,
                  "w2p": w2p_in, "w2s": w2s_in, "b1": b1_in, "b2": b2_in}

        def gv(slot, j):
            return gbuf[:, slot, j * 128:(j + 1) * 128]

        def run_ops(eng, name):
            for op in ops[name]:
                kind = op[0]
                if kind == "wait":
                    eng.wait_ge(sems[op[1]], op[2])
                elif kind == "dma_sb":
                    eng.dma_start(out=sb_map[op[1]][:], in_=in_map[op[1]][:]
                                  ).then_inc(dma_s, 16)
                elif kind == "iota":
                    eng.iota(iota_sb[:], pattern=[[1, 128]], base=0,
                             channel_multiplier=0,
                             allow_small_or_imprecise_dtypes=True)
                    eng.drain()
                elif kind == "ident":
                    eng.memset(ident_sb[:], 0.0)
                    eng.drain()
                    masks.make_identity(nc, ident_sb[:], nomemset=True)
                    eng.drain()
                    eng.memset(ident_sb[:1, :1], 1.0).then_inc(pl_s, 1)
                elif kind == "gather":
                    _, ph, k, g0, gsum, slot = op
                    if ph == "p1":
                        tbl = xg[int(trowbase[k]):int(trowbase[k + 1]), :]
                        idxs = idx1_sb
                    else:
                        t = p2t if ph == "p2" else h2t
                        tbl = t[pairbase[k]:pairbase[k + 1], :]
                        idxs = idx2_sb
                    eng.dma_gather(
                        gbuf[:, slot, :gsum * 128].rearrange(
                            "p (g f) -> p g f", f=128),
                        tbl,
                        idxs[:, g0 * 8:(g0 + gsum) * 8],
                        num_idxs=gsum * 128, num_idxs_reg=gsum * 128,
                        elem_size=128, single_packet=False,
                    ).then_inc(gt_sems[slot], 16)
                elif kind == "ohw":
                    _, g, vname, sh = op
                    vsb = {"val": val_sb, "vale": vale_sb, "valo": valo_sb}[vname]
                    slot = g % OHS if vname == "val" else (2 * g + sh) % OHS
                    sem = pl_s if vname == "valo" else v_s
                    eng.tensor_scalar(out=oh_sb[:, slot, :],
                                      in0=iota_sb[:],
                                      scalar1=dloc_sb[:, g:g + 1],
                                      scalar2=vsb[:, g:g + 1],
                                      op0=ALU.is_equal,
                                      op1=ALU.mult).then_inc(sem, 1)
                elif kind == "ohu":
                    g = op[1]
                    eng.tensor_scalar(out=oh_sb[:, g % OHS, :],
                                      in0=iota_sb[:],
                                      scalar1=dloc_sb[:, g:g + 1],
                                      scalar2=None,
                                      op0=ALU.is_equal).then_inc(pl_s, 1)
                elif kind == "agg1":
                    _, b, g, j, slot, first, last = op
                    eng.matmul(aggreg(b, 128), lhsT=gv(slot, j),
                               rhs=oh_sb[:, g % OHS, :], start=first,
                               stop=last).then_inc(p_s, 1)
                elif kind == "agg2":
                    _, b, g, j, slot, first, last = op
                    gvj = gv(slot, j)
                    eng.matmul(aggreg(b, 64),
                               lhsT=oh_sb[:, (2 * g) % OHS, :],
                               rhs=gvj[:, :64], start=first,
                               stop=False).then_inc(p_s, 1)
                    eng.matmul(aggreg(b, 64),
                               lhsT=oh_sb[:, (2 * g + 1) % OHS, :],
                               rhs=gvj[:, 64:], start=False,
                               stop=last).then_inc(p_s, 1)
                elif kind == "aggcopy":
                    b = op[1]
                    eng.activation(aggT_sb[:, b % 2, :], aggreg(b, 128),
                                   AF.Copy).then_inc(a_s, 1)
                elif kind == "h1mm":
                    b = op[1]
                    eng.matmul(h1b[:, :128], lhsT=w1p_sb[:],
                               rhs=aggT_sb[:, b % 2, :], start=True,
                               stop=False).then_inc(p_s, 1)
                    eng.matmul(h1b[:, :128], lhsT=w1s_sb[:],
                               rhs=xlT_sb[:, b * 128:(b + 1) * 128],
                               start=False, stop=True).then_inc(p_s, 1)
                elif kind == "h1relu":
                    b = op[1]
                    eng.activation(h1T_sb[:, b * 128:(b + 1) * 128],
                                   h1b[:, :128], AF.Relu, bias=b1_sb[:]
                                   ).then_inc(a_s, 1)
                elif kind == "p2mm":
                    b = op[1]
                    eng.matmul(p2s2b[:, :64],
                               lhsT=h1T_sb[:, b * 128:(b + 1) * 128],
                               rhs=w2p_sb[:], start=True, stop=True
                               ).then_inc(p_s, 1)
                    eng.matmul(p2s2b[:, 64:128],
                               lhsT=h1T_sb[:, b * 128:(b + 1) * 128],
                               rhs=w2s_sb[:], start=True, stop=True
                               ).then_inc(p_s, 1)
                elif kind == "p2cp":
                    b = op[1]
                    eng.activation(p2nm_sb[:, b, :], p2s2b[:, :64],
                                   AF.Copy).then_inc(a_s, 1)
                elif kind == "s2cp":
                    b = op[1]
                    eng.activation(s2_sb[:, b, :], p2s2b[:, 64:128],
                                   AF.Copy).then_inc(a_s, 1)
                elif kind == "p2wr":
                    b = op[1]
                    eng.dma_start(out=p2_loc[b * 128:(b + 1) * 128, :],
                                  in_=p2nm_sb[:, b, :]).then_inc(
                        wr_sems[piece_of_block[b]], 16)
                elif kind == "s2bias":
                    eng.tensor_tensor(
                        out=s2_sb[:], in0=s2_sb[:],
                        in1=b2_sb[:, None, :].to_broadcast([128, NB, 64]),
                        op=ALU.add).then_inc(v_s, 1)
                elif kind == "ag":
                    _, which, k = op
                    loc = p2_loc if which == "p2" else h2_loc
                    tab = p2t if which == "p2" else h2t
                    eng.collective_compute(
                        "AllGather", ALU.bypass,
                        replica_groups=[list(range(NCORES))],
                        ins=[loc[locbase[k]:locbase[k] + pn[k] * 128, :]],
                        outs=[tab[pairbase[k]:pairbase[k + 1], :]],
                    ).then_inc(cc_s, 1)
                elif kind == "h2add":
                    b = op[1]
                    eng.tensor_tensor(out=h2pre_sb[:, b % 2, :],
                                      in0=aggreg(b, 64),
                                      in1=s2_sb[:, b, :],
                                      op=ALU.add).then_inc(v_s, 1)
                elif kind == "h2relu":
                    b = op[1]
                    eng.activation(h2nm_sb[:, b, :], h2pre_sb[:, b % 2, :],
                                   AF.Relu).then_inc(a_s, 1)
                elif kind == "h2wr":
                    b = op[1]
                    eng.dma_start(out=h2_loc[b * 128:(b + 1) * 128, :],
                                  in_=h2nm_sb[:, b, :]).then_inc(
                        wr_sems[piece_of_block[b]], 16)
                elif kind == "p3tr":
                    _, g, wi, tb = op
                    eng.transpose(
                        out=aggps[tb][:].bitcast(BF16)[:, wi * 128:(wi + 1) * 128],
                        in_=oh_sb[:, g % OHS, :],
                        identity=ident_sb[:]).then_inc(p_s, 1)
                elif kind == "p3ocp":
                    _, w, nb = op
                    eng.activation(
                        osb_sb[:, w % 2, :nb * 128],
                        aggps[w % 2][:].bitcast(BF16)[:, :nb * 128],
                        AF.Copy).then_inc(a_s, 1)
                elif kind == "p3exp":
                    _, g, wi, eb, b = op
                    ebt = h1b if eb == 0 else p2s2b
                    eng.matmul(ebt[:, wi * 64:(wi + 1) * 64],
                               lhsT=osb_sb[:, eb, wi * 128:(wi + 1) * 128],
                               rhs=h2nm_sb[:, b, :], start=True, stop=True
                               ).then_inc(p_s, 1)
                elif kind == "p3mult":
                    _, w, gw0, nb, slot, g0s, hi = op
                    pr = slice(64, 128) if hi else slice(0, 64)
                    cr = slice(64, 128) if hi else slice(0, 64)
                    j0 = gw0 - g0s
                    ebt = h1b if w % 2 == 0 else p2s2b
                    eng.tensor_tensor(
                        out=prod_sb[pr, w % 2, :nb, :],
                        in0=ebt[pr, :nb * 64].rearrange(
                            "p (g f) -> p g f", f=64),
                        in1=gbuf[pr, slot, j0 * 128:(j0 + nb) * 128].rearrange(
                            "p (g f) -> p g f", f=128)[:, :, cr],
                        op=ALU.mult).then_inc(v_s, 1)
                elif kind == "p3red":
                    _, w, gw0, nb = op
                    eng.reduce_sum(out=dots_sb[:, gw0:gw0 + nb],
                                   in_=prod_sb[:, w % 2, :nb, :],
                                   axis=mybir.AxisListType.X).then_inc(v_s, 1)
                elif kind == "sigmoid":
                    eng.activation(dots_sb[:], dots_sb[:], AF.Sigmoid
                                   ).then_inc(a_s, 1)
                elif kind == "sxwr":
                    eng.dma_start(out=sx_out[:], in_=dots_sb[:]
                                  ).then_inc(dma_s, 16)
                else:
                    raise ValueError(kind)

        @block.sync
        def _(e):
            run_ops(e, "sp")

        @block.gpsimd
        def _(e):
            run_ops(e, "pool")

        @block.vector
        def _(e):
            run_ops(e, "dve")

        @block.scalar
        def _(e):
            run_ops(e, "act")

        @block.tensor
        def _(e):
            run_ops(e, "pe")

    nc.compile()
    return nc


# ---------------------------------------------------------------- host glue
def host_prep(X, edge_row, edge_col, edge_vals, W1p, b1p, W1s, b1s,
              W2p, b2p, W2s, b2s, plan):
    p = plan
    NP, NPc = p.NP, p.NPc
    Xp = np.zeros((NP, X.shape[1]), np.float32)
    Xp[: X.shape[0]] = X
    Xperm = Xp[p.perm]                       # (c,b,l)-ordered
    Xtab = Xp[p.node_of_trow]                # table-row ordered
    Xg = Xtab.astype(ml_dtypes.bfloat16)
    b1 = np.ascontiguousarray((b1p + b1s).astype(np.float32)[:, None])
    b2rep = np.ascontiguousarray(
        np.tile((b2p + b2s).astype(np.float32)[None, :], (128, 1)))
    in_maps = []
    for c in range(NCORES):
        in_maps.append({
            "xg": Xg,
            "xlT": np.ascontiguousarray(
                Xperm[c * NPc:(c + 1) * NPc].T).astype(ml_dtypes.bfloat16),
            "idx1": wrap_idx(p.idx1[c]),
            "idx2": wrap_idx(p.idx2[c]),
            "dloc": colmajor(p.dloc[c]),
            "val": colmajor(p.val[c]),
            "vale": colmajor(p.val[c]) * p.parmask[:, None],
            "valo": colmajor(p.val[c]) * (1.0 - p.parmask)[:, None],
            "w1p": np.ascontiguousarray(W1p).astype(ml_dtypes.bfloat16),
            "w1s": np.ascontiguousarray(W1s).astype(ml_dtypes.bfloat16),
            "w2p": np.ascontiguousarray(W2p).astype(ml_dtypes.bfloat16),
            "w2s": np.ascontiguousarray(W2s).astype(ml_dtypes.bfloat16),
            "b1": b1, "b2rep": b2rep,
        })
    return in_maps


def unpermute_sx(results, plan, n_edges):
    p = plan
    sx = np.empty(n_edges, np.float32)
    for c in range(NCORES):
        flat = results[c]["sx"].T.reshape(-1)
        m = p.core_of_edge[:n_edges] == c
        sx[m] = flat[p.slot_of_edge[m]]
    return sx


_CACHE = {}


def kernel(X, edge_row, edge_col, edge_vals,
           W_pass1, b_pass1, W_self1, b_self1,
           W_pass2, b_pass2, W_self2, b_self2):
    X = np.asarray(X, np.float32)
    er = np.asarray(edge_row).astype(np.int64)
    ec = np.asarray(edge_col).astype(np.int64)
    ev_ = np.asarray(edge_vals, np.float32)
    n_nodes, n_edges = X.shape[0], len(er)

    key = (n_nodes, n_edges, int(er[0]), int(ec[0]))
    if key not in _CACHE:
        plan = plan_graph(er, ec, ev_, n_nodes)
        nc = build(plan)
        _CACHE[key] = (plan, nc)
    plan, nc = _CACHE[key]

    in_maps = host_prep(X, er, ec, ev_,
                        np.asarray(W_pass1), np.asarray(b_pass1),
                        np.asarray(W_self1), np.asarray(b_self1),
                        np.asarray(W_pass2), np.asarray(b_pass2),
                        np.asarray(W_self2), np.asarray(b_self2), plan)
    res = run_bass_kernel_spmd(nc, in_maps, core_ids=list(range(NCORES)))
    return unpermute_sx(res.results, plan, n_edges)
